# revision 5
# baseline (speedup 1.0000x reference)
"""Trainium2 Bass kernel for nn_MultiHeadAttention_31542239822105.

Math (faithful to reference, incl. softmax over the QUERY axis):
  q = einsum('bsd,hde->bhse', x, Wq) + bq ; same k, v
  scores = q @ k^T * 1/sqrt(DH)          [B,H,Sq,Sk]
  probs  = softmax(scores, axis=2)       # over q (query axis!)
  ctx    = einsum('bhqk,bhke->bhqe', probs, v)
  out    = ctx.reshape(B,S,D) @ Wo + bo

Sharding: data-parallel over batch, 8 cores x 8 batch items. No collectives.

Per-core layout strategy (all matmul contraction dims land on partitions):
  - x is pre-transposed on the HOST to xT [D, tokens] so no on-chip transposes.
  - Q^T,K^T come out of the projection f-major ([feat, token]) with W as the
    stationary operand; V comes out token-major with xT as stationary.
  - scoresT[k,q] = K^T.T @ Q^T per head -> softmax over q is a FREE-axis
    reduction; exp+sum fused into the PSUM eviction on ScalarE (accum_out).
  - 1/denominator is folded into V rows (cheap: S*DH vs S*S elements).
  - ctxT[f,q] accumulates per head pair into one PSUM tile; output projection
    uses ctxT chunks as stationary -> token-major result, direct DMA out.
  - 1/sqrt(DH) folded into Wq/bq on the host.

Schedule (the perf-critical part): the ScalarE exp/accum chain (~830ns per
128x577 tile) is slower than the PE's scores+ctx work for the same tile
(~480ns), so a per-item serial stage order starves the PE during attention.
Instead the emission order software-pipelines ACROSS batch items: while
item b's attention drains on ScalarE, the PE executes interleaved "filler"
matmuls from item b+1's QK/V projections and item b-1's output projection.
Fillers are popped from a queue between the scores/ctx matmul pairs, so the
in-order PE stream always has independent work and stays in its fast clock
state (p-state ramps to 2.4 GHz only after ~3us of continuous execution).

Engine balance per item: PE ~65us, ScalarE(exp+accum) ~50us, DVE (all
PSUM evictions) ~15us, GpSimd (1/den folding into V) ~10us.

K^T and the scaled-V operands are zero-PADDED per head so the scores/ctx
lhsT is a full 128-partition operand (half-shape matmuls drop the PE out of
its fast clock); the zero halves live in persistent double-buffered tiles
that are memset once at kernel start, never per item.
"""

import sys
from collections import deque

if "/opt/trn_rl_repo" not in sys.path:
    sys.path.insert(0, "/opt/trn_rl_repo")

import numpy as np
import ml_dtypes

import concourse.bass as bass
import concourse.mybir as mybir
import concourse.tile as tile_mod
from concourse.vector_clock import ScopedClock
from concourse.bass_utils import run_bass_kernel_spmd

# ---------------------------------------------------------------- constants
B, S, D, H = 64, 577, 768, 12
DH = D // H          # 64
NCORES = 8
BC = B // NCORES     # 8 batch items per core
DC = D // 128        # 6 d-chunks
FC = D // 128        # 6 f-chunks per projection matrix
M_QK = 2 * FC        # 12 combined Q+K f-chunks
TT = (S + 127) // 128  # 5 token tiles (128,128,128,128,65)
S0 = 512             # PSUM-bank-sized free-dim split: 577 = 512 + 65
S1 = S - S0

BF16 = mybir.dt.bfloat16
F32 = mybir.dt.float32
nbf = ml_dtypes.bfloat16

_TILE_PATCHED = False
_CUR_NC = [None]


def _patch_tile_drain():
    """The walrus build here rejects >1 sync-wait per instruction
    ("Too many sync wait commands"). Two patches:
    1. post-legalize pass that moves extra waits onto single-wait nops
       inserted just before the offending instruction (same engine);
    2. the final SP Drain (emitted after legalize) gets the same split.
    """
    global _TILE_PATCHED
    if _TILE_PATCHED:
        return
    _TILE_PATCHED = True

    _orig_postorder = tile_mod.postorder_instruction_blocks

    def _split_multi_waits(ordered, nc):
        for bbname, insts in ordered.items():
            out = []
            n_split = 0
            for inst in insts:
                si = inst.sync_info
                if si is not None and len(si.on_wait) > 1:
                    waits = list(si.on_wait)
                    for w in waits[:-1]:
                        nop = mybir.InstNoOp(
                            name=nc.get_next_instruction_name(),
                            ins=[],
                            outs=[],
                            bass_is_fusable=False,
                        )
                        nop.engine = inst.engine
                        nop.sync_info = mybir.SyncInfo(on_wait=[w], on_update=[])
                        nc.register_instruction(nop, overwrite=True)
                        out.append(nop)
                        n_split += 1
                    inst.sync_info = mybir.SyncInfo(
                        on_wait=[waits[-1]], on_update=list(si.on_update)
                    )
                out.append(inst)
            ordered[bbname] = out
        return ordered

    def postorder_and_split(ordered, start_bb, postordered):
        # Runs post-sem-assignment, right before lowering: the only spot
        # where the final per-instruction waits are visible and editable.
        nc = _CUR_NC[0]
        _split_multi_waits(ordered, nc)
        return _orig_postorder(ordered, start_bb, postordered)

    tile_mod.postorder_instruction_blocks = postorder_and_split

    def _drain_and_barrier_split(self, tick_clock, wait_clock):
        nc = self.nc
        drain_inst = nc.sync.drain()
        wait_clock.add_sem_waits(
            drain_inst.ins, ScopedClock({None: tick_clock.global_clock})
        )
        si = drain_inst.ins.sync_info
        waits = list(si.on_wait)
        if len(waits) > 1:
            drain_inst.ins.sync_info = mybir.SyncInfo(
                on_wait=[waits[0]], on_update=list(si.on_update)
            )
            for w in waits[1:]:
                nop = nc.sync.nop(nofuse=True)
                nop.ins.sync_info = mybir.SyncInfo(on_wait=[w], on_update=[])
        nc.all_engine_barrier()
        assert self.sems is not None
        popped = nc._tile_sem_poison_stack.pop()
        assert popped is self._sem_poison
        nc.clear_and_free_semaphores(list(self.sems.allocated().values()))
        nc.all_engine_barrier()

    tile_mod.TileContext._drain_and_barrier = _drain_and_barrier_split


# ---------------------------------------------------------------- builder
def build_bass(bc=BC):
    """Emit the per-core kernel for `bc` batch items. Returns nc."""
    _patch_tile_drain()
    nc = bass.Bass()
    _CUR_NC[0] = nc

    xt_d = nc.declare_dram_parameter("xt", [DC, 128, bc, S], BF16, isOutput=False)
    wqk_d = nc.declare_dram_parameter("wqk", [128, M_QK, DC, 128], BF16, isOutput=False)
    wv_d = nc.declare_dram_parameter("wv", [128, DC, D], BF16, isOutput=False)
    wo_d = nc.declare_dram_parameter("wo", [128, FC, D], BF16, isOutput=False)
    bqk_d = nc.declare_dram_parameter("bqk", [128, M_QK], F32, isOutput=False)
    bvbc_d = nc.declare_dram_parameter("bvbc", [128, D], F32, isOutput=False)
    bobc_d = nc.declare_dram_parameter("bobc", [128, D], F32, isOutput=False)
    out_d = nc.declare_dram_parameter("out", [bc, S, D], F32, isOutput=True)

    AF = mybir.ActivationFunctionType

    with tile_mod.TileContext(nc) as tc:
        with (
            tc.tile_pool(name="singles", bufs=1) as singles,
            tc.tile_pool(name="xt", bufs=2) as xpool,
            tc.tile_pool(name="qk", bufs=2) as qkpool,
            tc.tile_pool(name="v", bufs=2) as vpool,
            tc.tile_pool(name="probs", bufs=4) as ppool,
            tc.tile_pool(name="den", bufs=3) as dpool,
            tc.tile_pool(name="ctx", bufs=2) as cpool,
            tc.tile_pool(name="ot", bufs=3) as opool,
            tc.tile_pool(name="pss", bufs=2, space="PSUM") as pss,
            tc.tile_pool(name="psc", bufs=1, space="PSUM") as psc,
            tc.tile_pool(name="psf", bufs=1, space="PSUM") as psf,
        ):
            # -------- resident weights / biases.  wqk is split per d-chunk so
            # the first projection matmul only waits on its own slice.
            wqk = singles.tile([128, M_QK, DC, 128], BF16, tag="wqk")
            for dc in range(DC):
                nc.sync.dma_start(out=wqk[:, :, dc, :], in_=wqk_d[:, :, dc, :])
            bqk = singles.tile([128, M_QK], F32, tag="bqk")
            nc.sync.dma_start(out=bqk, in_=bqk_d[:])
            wv = singles.tile([128, DC, D], BF16, tag="wv")
            nc.sync.dma_start(out=wv, in_=wv_d[:])
            wo = singles.tile([128, FC, D], BF16, tag="wo")
            nc.sync.dma_start(out=wo, in_=wo_d[:])
            bvbc = singles.tile([128, D], F32, tag="bvbc")
            nc.sync.dma_start(out=bvbc, in_=bvbc_d[:])
            bobc = singles.tile([128, D], F32, tag="bobc")
            nc.sync.dma_start(out=bobc, in_=bobc_d[:])

            # Persistent zero-padded operand buffers (double-buffered by item
            # parity for ktz, by head parity for vsz).  The zero halves are
            # written ONCE here and never touched again.
            ktzs = [
                singles.tile([128, FC, 2, S], BF16, tag=f"ktz{i}", name=f"ktz{i}")
                for i in range(2)
            ]
            vszs = [
                singles.tile([128, TT, 128], BF16, tag=f"vsz{i}", name=f"vsz{i}")
                for i in range(2)
            ]
            for i in range(2):
                for mk in range(FC):
                    nc.vector.memset(ktzs[i][64:128, mk, 0, :], 0.0)
                    nc.vector.memset(ktzs[i][0:64, mk, 1, :], 0.0)
            nc.vector.memset(vszs[0][:, :, 64:128], 0.0)
            nc.vector.memset(vszs[1][:, :, 0:64], 0.0)

            st = {}  # cross-stage tile handles: ('xt'|'qk'|'v'|'ctx', b)

            # ---------------- filler-step generators (one PE matmul pair
            # or one DMA batch per step; evictions ride along).
            def load_steps(b):
                def go():
                    xt = xpool.tile([128, DC, S], BF16, tag="xt", name="xt")
                    st[("xt", b)] = xt
                    for dc in range(DC):
                        nc.sync.dma_start(out=xt[:, dc, :], in_=xt_d[dc, :, b, :])

                return [go]

            def qk_steps(b):
                steps = []
                box = {}
                for m in range(M_QK):
                    for dc in range(DC):
                        def go(m=m, dc=dc):
                            xt = st[("xt", b)]
                            if m == 0 and dc == 0:
                                st[("qk", b)] = qkpool.tile(
                                    [128, FC, S], BF16, tag="qk", name="qk"
                                )
                            if dc == 0:
                                box["ps"] = psf.tile(
                                    [128, D], F32, tag="psf", name="psfq"
                                )
                            ps = box["ps"]
                            stt, spp = dc == 0, dc == DC - 1
                            nc.tensor.matmul(
                                ps[:, 0:S0], lhsT=wqk[:, m, dc, :],
                                rhs=xt[:, dc, 0:S0], start=stt, stop=spp)
                            nc.tensor.matmul(
                                ps[:, S0:S], lhsT=wqk[:, m, dc, :],
                                rhs=xt[:, dc, S0:S], start=stt, stop=spp)
                            if spp:
                                if m < FC:
                                    nc.vector.tensor_scalar_add(
                                        st[("qk", b)][:, m, :], ps[:, 0:S],
                                        bqk[:, m : m + 1])
                                else:
                                    mk = m - FC
                                    ktz = ktzs[b % 2]
                                    nc.vector.tensor_scalar_add(
                                        ktz[0:64, mk, 0, :], ps[0:64, 0:S],
                                        bqk[0:64, m : m + 1])
                                    nc.vector.tensor_scalar_add(
                                        ktz[64:128, mk, 1, :], ps[64:128, 0:S],
                                        bqk[64:128, m : m + 1])

                        steps.append(go)
                return steps

            def v_steps(b, pool=None):
                steps = []
                box = {}
                for tt in range(TT):
                    for dc in range(DC):
                        def go(tt=tt, dc=dc):
                            xt = st[("xt", b)]
                            tsz = min(128, S - tt * 128)
                            t0 = tt * 128
                            if tt == 0 and dc == 0:
                                st[("v", b)] = vpool.tile(
                                    [128, TT, D], BF16, tag="v", name="v"
                                )
                            if dc == 0:
                                p = pool if pool is not None else psf
                                tg = "pss" if pool is not None else "psf"
                                box["ps"] = p.tile(
                                    [128, D], F32, tag=tg, name="psfv"
                                )
                            ps = box["ps"]
                            stt, spp = dc == 0, dc == DC - 1
                            nc.tensor.matmul(
                                ps[:tsz, 0:S0], lhsT=xt[:, dc, t0 : t0 + tsz],
                                rhs=wv[:, dc, 0:S0], start=stt, stop=spp)
                            nc.tensor.matmul(
                                ps[:tsz, S0:D], lhsT=xt[:, dc, t0 : t0 + tsz],
                                rhs=wv[:, dc, S0:D], start=stt, stop=spp)
                            if spp:
                                nc.vector.tensor_add(
                                    st[("v", b)][:tsz, tt, :], ps[:tsz, 0:D],
                                    bvbc[:tsz])

                        steps.append(go)
                return steps

            def o_steps(b):
                steps = []
                box = {}
                for tt in range(TT):
                    for fc in range(FC):
                        def go(tt=tt, fc=fc):
                            ctxT = st[("ctx", b)]
                            tsz = min(128, S - tt * 128)
                            t0 = tt * 128
                            if fc == 0:
                                box["ps"] = psf.tile(
                                    [128, D], F32, tag="psf", name="psfo"
                                )
                            ps = box["ps"]
                            stt, spp = fc == 0, fc == FC - 1
                            nc.tensor.matmul(
                                ps[:tsz, 0:S0], lhsT=ctxT[:, fc, t0 : t0 + tsz],
                                rhs=wo[:, fc, 0:S0], start=stt, stop=spp)
                            nc.tensor.matmul(
                                ps[:tsz, S0:D], lhsT=ctxT[:, fc, t0 : t0 + tsz],
                                rhs=wo[:, fc, S0:D], start=stt, stop=spp)
                            if spp:
                                ot = opool.tile([128, D], F32, tag="ot", name="ot")
                                nc.vector.tensor_add(
                                    ot[:tsz], ps[:tsz, 0:D], bobc[:tsz])
                                nc.sync.dma_start(
                                    out=out_d[b, t0 : t0 + tsz, :], in_=ot[:tsz])

                        steps.append(go)
                return steps

            def run_all(steps):
                for s in steps:
                    s()

            # ---------------- attention for item b, fillers interleaved
            def emit_attention(b, fillers):
                qk = st[("qk", b)]
                v = st[("v", b)]
                ktz = ktzs[b % 2]
                ctxT = cpool.tile([128, FC, S], BF16, tag="ctx", name="ctx")
                st[("ctx", b)] = ctxT

                nslots = H + 2
                # pop points: one after each kc iteration + one per slot end
                points = {"left": nslots * (TT + 1)}

                def pop_fill():
                    points["left"] -= 1
                    if not fillers:
                        return
                    n = (len(fillers) + max(points["left"], 1) - 1) // max(
                        points["left"], 1
                    )
                    for _ in range(min(n, len(fillers))):
                        fillers.popleft()()

                hstate = {}  # h -> probs tile
                box = {"psc": None}
                for hs in range(nslots):
                    h_s = hs if hs < H else None
                    h_c = hs - 2 if hs >= 2 else None
                    if h_s is not None:
                        m, j = h_s // 2, h_s % 2
                        probs = ppool.tile(
                            [128, TT, S], BF16, tag="probs", name="probs"
                        )
                        den = dpool.tile([128, TT], F32, tag="den", name="den")
                        nc.vector.memset(den, 1.0)
                        hstate[h_s] = probs
                    if h_c is not None:
                        probs_c = hstate.pop(h_c) if h_c % 2 == 1 else hstate[h_c]
                        if h_c % 2 == 0:
                            box["psc"] = psc.tile(
                                [128, D], F32, tag="psc", name="pscx"
                            )
                        vsz_c = vszs[h_c % 2]
                    for kc in range(TT):
                        ksz = min(128, S - kc * 128)
                        k0 = kc * 128
                        if h_s is not None:
                            ps = pss.tile([128, D], F32, tag="pss", name="pssc")
                            nc.tensor.matmul(
                                ps[:ksz, 0:S0],
                                lhsT=ktz[:, m, j, k0 : k0 + ksz],
                                rhs=qk[:, m, 0:S0], start=True, stop=True)
                            nc.tensor.matmul(
                                ps[:ksz, S0:S],
                                lhsT=ktz[:, m, j, k0 : k0 + ksz],
                                rhs=qk[:, m, S0:S], start=True, stop=True)
                            # exp + row-sum (over q) fused in the eviction
                            nc.scalar.activation(
                                probs[:ksz, kc, :], ps[:ksz, 0:S], AF.Exp,
                                accum_out=den[:ksz, kc : kc + 1])
                        if h_c is not None:
                            stt = (h_c % 2 == 0) and kc == 0
                            spp = (h_c % 2 == 1) and kc == TT - 1
                            nc.tensor.matmul(
                                box["psc"][:, 0:S0], lhsT=vsz_c[:ksz, kc, :],
                                rhs=probs_c[:ksz, kc, 0:S0], start=stt, stop=spp)
                            nc.tensor.matmul(
                                box["psc"][:, S0:S], lhsT=vsz_c[:ksz, kc, :],
                                rhs=probs_c[:ksz, kc, S0:S], start=stt, stop=spp)
                        pop_fill()
                    if h_s is not None:
                        # fold 1/denominator into this head's V rows (GpSimd;
                        # writes only the live half of the persistent buffer)
                        po = j * 64
                        rd = dpool.tile([128, TT], F32, tag="rd", name="rd")
                        nc.vector.reciprocal(rd, den)
                        vsz = vszs[j]
                        for kc in range(TT):
                            ksz = min(128, S - kc * 128)
                            nc.gpsimd.tensor_scalar_mul(
                                vsz[:ksz, kc, po : po + 64],
                                v[:ksz, kc, h_s * DH : (h_s + 1) * DH],
                                rd[:ksz, kc : kc + 1])
                    if h_c is not None and h_c % 2 == 1:
                        nc.vector.tensor_copy(
                            ctxT[:, h_c // 2, :], box["psc"][:, 0:S])
                    pop_fill()
                # drain any leftover fillers
                while fillers:
                    fillers.popleft()()

            # ---------------- software-pipelined item loop.  Prologue:
            # QK chunks go to the psf pool, V chunks to the (not yet used)
            # pss pool, alternating per chunk so the single-buffered psf
            # eviction latency hides under the other stream's matmuls.
            run_all(load_steps(0))
            if bc > 1:
                run_all(load_steps(1))
            qs, vs = qk_steps(0), v_steps(0, pool=pss)
            qchunks = [qs[i : i + DC] for i in range(0, len(qs), DC)]
            vchunks = [vs[i : i + DC] for i in range(0, len(vs), DC)]
            order = []
            for i in range(max(len(qchunks), len(vchunks))):
                if i < len(qchunks):
                    order.extend(qchunks[i])
                if i < len(vchunks):
                    order.extend(vchunks[i])
            run_all(order)
            for b in range(bc):
                fillers = deque()
                if b + 2 < bc:
                    fillers.extend(load_steps(b + 2))
                if b >= 1:
                    fillers.extend(o_steps(b - 1))
                if b + 1 < bc:
                    fillers.extend(qk_steps(b + 1))
                    fillers.extend(v_steps(b + 1))
                emit_attention(b, fillers)
            run_all(o_steps(bc - 1))

    return nc


# ---------------------------------------------------------------- host prep
def _prep_shared(Wq, bq, Wk, bk, Wv, bv, Wo, bo):
    """Build the per-core-identical weight operands."""
    scale = np.float32(1.0 / np.sqrt(DH))
    wqf = (Wq.astype(np.float32) * scale).transpose(1, 0, 2).reshape(D, D)
    wkf = Wk.astype(np.float32).transpose(1, 0, 2).reshape(D, D)
    wvf = Wv.astype(np.float32).transpose(1, 0, 2).reshape(D, D)

    def chunk4(wf):  # [d, f] -> [di, m, dc, fi]
        return wf.reshape(DC, 128, FC, 128).transpose(1, 2, 0, 3)

    wqk = np.concatenate([chunk4(wqf), chunk4(wkf)], axis=1)  # [128, 12, 6, 128]
    wv3 = wvf.reshape(DC, 128, D).transpose(1, 0, 2)          # [128, 6, 768]
    wo3 = Wo.astype(np.float32).reshape(FC, 128, D).transpose(1, 0, 2)

    bqf = (bq.astype(np.float32) * scale).reshape(D)
    bkf = bk.astype(np.float32).reshape(D)
    bqk = np.concatenate(
        [bqf.reshape(FC, 128), bkf.reshape(FC, 128)], axis=0
    ).T.copy()                                                # [128, 12]
    bvbc = np.broadcast_to(bv.astype(np.float32).reshape(D), (128, D)).copy()
    bobc = np.broadcast_to(bo.astype(np.float32).reshape(D), (128, D)).copy()

    return {
        "wqk": np.ascontiguousarray(wqk).astype(nbf),
        "wv": np.ascontiguousarray(wv3).astype(nbf),
        "wo": np.ascontiguousarray(wo3).astype(nbf),
        "bqk": np.ascontiguousarray(bqk),
        "bvbc": bvbc,
        "bobc": bobc,
    }


_NC_CACHE = {}


def kernel(x, Wq, bq, Wk, bk, Wv, bv, Wo, bo):
    x = np.asarray(x, dtype=np.float32)
    shared = _prep_shared(
        np.asarray(Wq), np.asarray(bq), np.asarray(Wk), np.asarray(bk),
        np.asarray(Wv), np.asarray(bv), np.asarray(Wo), np.asarray(bo))

    in_maps = []
    for c in range(NCORES):
        xc = x[c * BC : (c + 1) * BC]                    # [BC, S, D]
        xt = xc.transpose(2, 0, 1)                       # [D, BC, S]
        xt = xt.reshape(DC, 128, BC, S).astype(nbf)
        m = dict(shared)
        m["xt"] = np.ascontiguousarray(xt)
        in_maps.append(m)

    if "nc" not in _NC_CACHE:
        _NC_CACHE["nc"] = build_bass()
    nc = _NC_CACHE["nc"]

    res = run_bass_kernel_spmd(nc, in_maps, core_ids=list(range(NCORES)))
    out = np.concatenate([res.results[c]["out"] for c in range(NCORES)], axis=0)
    return out.astype(np.float32)


if __name__ == "__main__":
    rng = np.random.default_rng(0)
    ins = {
        "x": rng.standard_normal((B, S, D), dtype=np.float32),
        "Wq": rng.standard_normal((H, D, DH), dtype=np.float32) * 0.02,
        "bq": np.zeros((H, DH), np.float32),
        "Wk": rng.standard_normal((H, D, DH), dtype=np.float32) * 0.02,
        "bk": np.zeros((H, DH), np.float32),
        "Wv": rng.standard_normal((H, D, DH), dtype=np.float32) * 0.02,
        "bv": np.zeros((H, DH), np.float32),
        "Wo": rng.standard_normal((D, D), dtype=np.float32) * 0.02,
        "bo": np.zeros((D,), np.float32),
    }
    o = kernel(**ins)
    print("out", o.shape, o.dtype, float(np.abs(o).max()))


# revision 10
# speedup vs baseline: 1.0119x; 1.0119x over previous
"""Trainium2 Bass kernel for nn_MultiHeadAttention_31542239822105.

Math (faithful to reference, incl. softmax over the QUERY axis):
  q = einsum('bsd,hde->bhse', x, Wq) + bq ; same k, v
  scores = q @ k^T * 1/sqrt(DH)          [B,H,Sq,Sk]
  probs  = softmax(scores, axis=2)       # over q (query axis!)
  ctx    = einsum('bhqk,bhke->bhqe', probs, v)
  out    = ctx.reshape(B,S,D) @ Wo + bo

Sharding: data-parallel over batch, 8 cores x 8 batch items. No collectives.

Per-core layout strategy (all matmul contraction dims land on partitions):
  - x is pre-transposed on the HOST to xT [D, tokens] so no on-chip transposes.
  - Q^T,K^T come out of the projection f-major ([feat, token]) with W as the
    stationary operand; V comes out token-major with xT as stationary.
  - scoresT[k,q] = K^T.T @ Q^T per head -> softmax over q is a FREE-axis
    reduction; exp+sum fused into the PSUM eviction on ScalarE (accum_out).
  - 1/denominator is folded into V rows (cheap: S*DH vs S*S elements).
  - ctxT[f,q] accumulates per head pair into one PSUM tile; output projection
    uses ctxT chunks as stationary -> token-major result, direct DMA out.
  - 1/sqrt(DH) folded into Wq/bq on the host.

Schedule (the perf-critical part): the ScalarE exp/accum chain (~830ns per
128x577 tile) is slower than the PE's scores+ctx work for the same tile
(~480ns), so a per-item serial stage order starves the PE during attention.
Instead the emission order software-pipelines ACROSS batch items: while
item b's attention drains on ScalarE, the PE executes interleaved "filler"
matmuls from item b+1's QK/V projections and item b-1's output projection.
Fillers are popped from a queue between the scores/ctx matmul pairs, so the
in-order PE stream always has independent work and stays in its fast clock
state (p-state ramps to 2.4 GHz only after ~3us of continuous execution).

Engine balance per item: PE ~65us, ScalarE(exp+accum) ~50us, DVE (all
PSUM evictions) ~15us, GpSimd (1/den folding into V) ~10us.

K^T and the scaled-V operands are zero-PADDED per head so the scores/ctx
lhsT is a full 128-partition operand (half-shape matmuls drop the PE out of
its fast clock); the zero halves live in persistent double-buffered tiles
that are memset once at kernel start, never per item.
"""

import sys
from collections import deque

if "/opt/trn_rl_repo" not in sys.path:
    sys.path.insert(0, "/opt/trn_rl_repo")

import numpy as np
import ml_dtypes

import concourse.bass as bass
import concourse.mybir as mybir
import concourse.tile as tile_mod
from concourse.vector_clock import ScopedClock
from concourse.bass_utils import run_bass_kernel_spmd

# ---------------------------------------------------------------- constants
B, S, D, H = 64, 577, 768, 12
DH = D // H          # 64
NCORES = 8
BC = B // NCORES     # 8 batch items per core
DC = D // 128        # 6 d-chunks
FC = D // 128        # 6 f-chunks per projection matrix
M_QK = 2 * FC        # 12 combined Q+K f-chunks
TT = (S + 127) // 128  # 5 token tiles (128,128,128,128,65)
S0 = 512             # PSUM-bank-sized free-dim split: 577 = 512 + 65
S1 = S - S0

BF16 = mybir.dt.bfloat16
F32 = mybir.dt.float32
nbf = ml_dtypes.bfloat16

_TILE_PATCHED = False
_CUR_NC = [None]


def _patch_tile_drain():
    """The walrus build here rejects >1 sync-wait per instruction
    ("Too many sync wait commands"). Two patches:
    1. post-legalize pass that moves extra waits onto single-wait nops
       inserted just before the offending instruction (same engine);
    2. the final SP Drain (emitted after legalize) gets the same split.
    """
    global _TILE_PATCHED
    if _TILE_PATCHED:
        return
    _TILE_PATCHED = True

    _orig_postorder = tile_mod.postorder_instruction_blocks

    def _split_multi_waits(ordered, nc):
        for bbname, insts in ordered.items():
            out = []
            n_split = 0
            for inst in insts:
                si = inst.sync_info
                if si is not None and len(si.on_wait) > 1:
                    waits = list(si.on_wait)
                    for w in waits[:-1]:
                        nop = mybir.InstNoOp(
                            name=nc.get_next_instruction_name(),
                            ins=[],
                            outs=[],
                            bass_is_fusable=False,
                        )
                        nop.engine = inst.engine
                        nop.sync_info = mybir.SyncInfo(on_wait=[w], on_update=[])
                        nc.register_instruction(nop, overwrite=True)
                        out.append(nop)
                        n_split += 1
                    inst.sync_info = mybir.SyncInfo(
                        on_wait=[waits[-1]], on_update=list(si.on_update)
                    )
                out.append(inst)
            ordered[bbname] = out
        return ordered

    def postorder_and_split(ordered, start_bb, postordered):
        # Runs post-sem-assignment, right before lowering: the only spot
        # where the final per-instruction waits are visible and editable.
        nc = _CUR_NC[0]
        _split_multi_waits(ordered, nc)
        return _orig_postorder(ordered, start_bb, postordered)

    tile_mod.postorder_instruction_blocks = postorder_and_split

    def _drain_and_barrier_split(self, tick_clock, wait_clock):
        nc = self.nc
        drain_inst = nc.sync.drain()
        wait_clock.add_sem_waits(
            drain_inst.ins, ScopedClock({None: tick_clock.global_clock})
        )
        si = drain_inst.ins.sync_info
        waits = list(si.on_wait)
        if len(waits) > 1:
            drain_inst.ins.sync_info = mybir.SyncInfo(
                on_wait=[waits[0]], on_update=list(si.on_update)
            )
            for w in waits[1:]:
                nop = nc.sync.nop(nofuse=True)
                nop.ins.sync_info = mybir.SyncInfo(on_wait=[w], on_update=[])
        nc.all_engine_barrier()
        assert self.sems is not None
        popped = nc._tile_sem_poison_stack.pop()
        assert popped is self._sem_poison
        nc.clear_and_free_semaphores(list(self.sems.allocated().values()))
        nc.all_engine_barrier()

    tile_mod.TileContext._drain_and_barrier = _drain_and_barrier_split


# ---------------------------------------------------------------- builder
def build_bass(bc=BC):
    """Emit the per-core kernel for `bc` batch items. Returns nc."""
    _patch_tile_drain()
    nc = bass.Bass()
    _CUR_NC[0] = nc

    xt_d = nc.declare_dram_parameter("xt", [DC, 128, bc, S], BF16, isOutput=False)
    wqk_d = nc.declare_dram_parameter("wqk", [128, M_QK, DC, 128], BF16, isOutput=False)
    wv_d = nc.declare_dram_parameter("wv", [128, DC, D], BF16, isOutput=False)
    wo_d = nc.declare_dram_parameter("wo", [128, FC, D], BF16, isOutput=False)
    bqk_d = nc.declare_dram_parameter("bqk", [128, M_QK], F32, isOutput=False)
    bvbc_d = nc.declare_dram_parameter("bvbc", [128, D], F32, isOutput=False)
    bobc_d = nc.declare_dram_parameter("bobc", [128, D], F32, isOutput=False)
    out_d = nc.declare_dram_parameter("out", [bc, S, D], F32, isOutput=True)

    AF = mybir.ActivationFunctionType

    with tile_mod.TileContext(nc) as tc:
        with (
            tc.tile_pool(name="singles", bufs=1) as singles,
            tc.tile_pool(name="xt", bufs=3) as xpool,
            tc.tile_pool(name="qk", bufs=2) as qkpool,
            tc.tile_pool(name="v", bufs=2) as vpool,
            tc.tile_pool(name="probs", bufs=4) as ppool,
            tc.tile_pool(name="den", bufs=3) as dpool,
            tc.tile_pool(name="ctx", bufs=2) as cpool,
            tc.tile_pool(name="ot", bufs=3) as opool,
            tc.tile_pool(name="pss", bufs=1, space="PSUM") as pss,
            tc.tile_pool(name="psc", bufs=1, space="PSUM") as psc,
            tc.tile_pool(name="psf", bufs=2, space="PSUM") as psf,
        ):
            # -------- resident weights / biases.  wqk is split per d-chunk so
            # the first projection matmul only waits on its own slice.
            wqk = singles.tile([128, M_QK, DC, 128], BF16, tag="wqk")
            for dc in range(DC):
                nc.sync.dma_start(out=wqk[:, :, dc, :], in_=wqk_d[:, :, dc, :])
            bqk = singles.tile([128, M_QK], F32, tag="bqk")
            nc.sync.dma_start(out=bqk, in_=bqk_d[:])
            wv = singles.tile([128, DC, D], BF16, tag="wv")
            nc.sync.dma_start(out=wv, in_=wv_d[:])
            wo = singles.tile([128, FC, D], BF16, tag="wo")
            nc.sync.dma_start(out=wo, in_=wo_d[:])
            bvbc = singles.tile([128, D], F32, tag="bvbc")
            nc.sync.dma_start(out=bvbc, in_=bvbc_d[:])
            bobc = singles.tile([128, D], F32, tag="bobc")
            nc.sync.dma_start(out=bobc, in_=bobc_d[:])

            # Persistent zero-padded operand buffers (double-buffered by item
            # parity for ktz, by head parity for vsz).  The zero halves are
            # written ONCE here and never touched again.
            ktzs = [
                singles.tile([128, FC, 2, S], BF16, tag=f"ktz{i}", name=f"ktz{i}")
                for i in range(2)
            ]
            vszs = [
                singles.tile([128, TT, 128], BF16, tag=f"vsz{i}", name=f"vsz{i}")
                for i in range(2)
            ]
            for i in range(2):
                for mk in range(FC):
                    nc.vector.memset(ktzs[i][64:128, mk, 0, :], 0.0)
                    nc.vector.memset(ktzs[i][0:64, mk, 1, :], 0.0)
            nc.vector.memset(vszs[0][:, :, 64:128], 0.0)
            nc.vector.memset(vszs[1][:, :, 0:64], 0.0)

            st = {}  # cross-stage tile handles: ('xt'|'qk'|'v'|'ctx', b)

            # ---------------- filler-step generators (one PE matmul pair
            # or one DMA batch per step; evictions ride along).
            def load_steps(b):
                def go():
                    xt = xpool.tile([128, DC, S], BF16, tag="xt", name="xt")
                    st[("xt", b)] = xt
                    for dc in range(DC):
                        nc.sync.dma_start(out=xt[:, dc, :], in_=xt_d[dc, :, b, :])

                return [go]

            def qk_steps(b, ms=None):
                steps = []
                box = {}
                for m in ms if ms is not None else range(M_QK):
                    for dc in range(DC):
                        def go(m=m, dc=dc):
                            xt = st[("xt", b)]
                            if ("qk", b) not in st:
                                st[("qk", b)] = qkpool.tile(
                                    [128, FC, S], BF16, tag="qk", name="qk"
                                )
                            if dc == 0:
                                box["ps"] = psf.tile(
                                    [128, D], F32, tag="psf", name="psfq"
                                )
                            ps = box["ps"]
                            stt, spp = dc == 0, dc == DC - 1
                            nc.tensor.matmul(
                                ps[:, 0:S0], lhsT=wqk[:, m, dc, :],
                                rhs=xt[:, dc, 0:S0], start=stt, stop=spp)
                            nc.tensor.matmul(
                                ps[:, S0:S], lhsT=wqk[:, m, dc, :],
                                rhs=xt[:, dc, S0:S], start=stt, stop=spp)
                            if spp:
                                if m < FC:
                                    nc.vector.tensor_scalar_add(
                                        st[("qk", b)][:, m, :], ps[:, 0:S],
                                        bqk[:, m : m + 1])
                                else:
                                    mk = m - FC
                                    ktz = ktzs[b % 2]
                                    nc.vector.tensor_scalar_add(
                                        ktz[0:64, mk, 0, :], ps[0:64, 0:S],
                                        bqk[0:64, m : m + 1])
                                    nc.vector.tensor_scalar_add(
                                        ktz[64:128, mk, 1, :], ps[64:128, 0:S],
                                        bqk[64:128, m : m + 1])

                        steps.append(go)
                return steps

            def v_steps(b, pool=None):
                steps = []
                box = {}
                for tt in range(TT):
                    for dc in range(DC):
                        def go(tt=tt, dc=dc):
                            xt = st[("xt", b)]
                            tsz = min(128, S - tt * 128)
                            t0 = tt * 128
                            if tt == 0 and dc == 0:
                                st[("v", b)] = vpool.tile(
                                    [128, TT, D], BF16, tag="v", name="v"
                                )
                            if dc == 0:
                                p = pool if pool is not None else psf
                                tg = "pss" if pool is not None else "psf"
                                box["ps"] = p.tile(
                                    [128, D], F32, tag=tg, name="psfv"
                                )
                            ps = box["ps"]
                            stt, spp = dc == 0, dc == DC - 1
                            nc.tensor.matmul(
                                ps[:tsz, 0:S0], lhsT=xt[:, dc, t0 : t0 + tsz],
                                rhs=wv[:, dc, 0:S0], start=stt, stop=spp)
                            nc.tensor.matmul(
                                ps[:tsz, S0:D], lhsT=xt[:, dc, t0 : t0 + tsz],
                                rhs=wv[:, dc, S0:D], start=stt, stop=spp)
                            if spp:
                                nc.vector.tensor_add(
                                    st[("v", b)][:tsz, tt, :], ps[:tsz, 0:D],
                                    bvbc[:tsz])

                        steps.append(go)
                return steps

            def o_steps(b):
                steps = []
                box = {}
                for tt in range(TT):
                    for fc in range(FC):
                        def go(tt=tt, fc=fc):
                            ctxT = st[("ctx", b)]
                            tsz = min(128, S - tt * 128)
                            t0 = tt * 128
                            if fc == 0:
                                box["ps"] = psf.tile(
                                    [128, D], F32, tag="psf", name="psfo"
                                )
                            ps = box["ps"]
                            stt, spp = fc == 0, fc == FC - 1
                            nc.tensor.matmul(
                                ps[:tsz, 0:S0], lhsT=ctxT[:, fc, t0 : t0 + tsz],
                                rhs=wo[:, fc, 0:S0], start=stt, stop=spp)
                            nc.tensor.matmul(
                                ps[:tsz, S0:D], lhsT=ctxT[:, fc, t0 : t0 + tsz],
                                rhs=wo[:, fc, S0:D], start=stt, stop=spp)
                            if spp:
                                ot = opool.tile([128, D], F32, tag="ot", name="ot")
                                nc.vector.tensor_add(
                                    ot[:tsz], ps[:tsz, 0:D], bobc[:tsz])
                                nc.sync.dma_start(
                                    out=out_d[b, t0 : t0 + tsz, :], in_=ot[:tsz])

                        steps.append(go)
                return steps

            def run_all(steps):
                for s in steps:
                    s()

            # ---------------- attention for item b, fillers interleaved
            def emit_attention(b, fillers):
                qk = st[("qk", b)]
                v = st[("v", b)]
                ktz = ktzs[b % 2]
                ctxT = cpool.tile([128, FC, S], BF16, tag="ctx", name="ctx")
                st[("ctx", b)] = ctxT

                nslots = H + 2
                # pop points: one after each kc iteration + one per slot end
                points = {"left": nslots * (TT + 1)}

                def pop_fill():
                    points["left"] -= 1
                    if not fillers:
                        return
                    n = (len(fillers) + max(points["left"], 1) - 1) // max(
                        points["left"], 1
                    )
                    for _ in range(min(n, len(fillers))):
                        fillers.popleft()()

                hstate = {}  # h -> probs tile
                box = {"psc": None}
                for hs in range(nslots):
                    h_s = hs if hs < H else None
                    h_c = hs - 2 if hs >= 2 else None
                    if h_s is not None:
                        m, j = h_s // 2, h_s % 2
                        probs = ppool.tile(
                            [128, TT, S], BF16, tag="probs", name="probs"
                        )
                        den = dpool.tile([128, TT], F32, tag="den", name="den")
                        nc.vector.memset(den, 1.0)
                        hstate[h_s] = probs
                    if h_c is not None:
                        probs_c = hstate.pop(h_c) if h_c % 2 == 1 else hstate[h_c]
                        if h_c % 2 == 0:
                            box["psc"] = psc.tile(
                                [128, D], F32, tag="psc", name="pscx"
                            )
                        vsz_c = vszs[h_c % 2]
                    for kc in range(TT):
                        ksz = min(128, S - kc * 128)
                        k0 = kc * 128
                        if h_s is not None:
                            ps = pss.tile([128, D], F32, tag="pss", name="pssc")
                            nc.tensor.matmul(
                                ps[:ksz, 0:S0],
                                lhsT=ktz[:, m, j, k0 : k0 + ksz],
                                rhs=qk[:, m, 0:S0], start=True, stop=True)
                            nc.tensor.matmul(
                                ps[:ksz, S0:S],
                                lhsT=ktz[:, m, j, k0 : k0 + ksz],
                                rhs=qk[:, m, S0:S], start=True, stop=True)
                            # exp + row-sum (over q) fused in the eviction
                            nc.scalar.activation(
                                probs[:ksz, kc, :], ps[:ksz, 0:S], AF.Exp,
                                accum_out=den[:ksz, kc : kc + 1])
                        if h_c is not None:
                            stt = (h_c % 2 == 0) and kc == 0
                            spp = (h_c % 2 == 1) and kc == TT - 1
                            nc.tensor.matmul(
                                box["psc"][:, 0:S0], lhsT=vsz_c[:ksz, kc, :],
                                rhs=probs_c[:ksz, kc, 0:S0], start=stt, stop=spp)
                            nc.tensor.matmul(
                                box["psc"][:, S0:S], lhsT=vsz_c[:ksz, kc, :],
                                rhs=probs_c[:ksz, kc, S0:S], start=stt, stop=spp)
                        pop_fill()
                    if h_s is not None:
                        # fold 1/denominator into this head's V rows (GpSimd;
                        # writes only the live half of the persistent buffer)
                        po = j * 64
                        rd = dpool.tile([128, TT], F32, tag="rd", name="rd")
                        nc.vector.reciprocal(rd, den)
                        vsz = vszs[j]
                        for kc in range(TT):
                            ksz = min(128, S - kc * 128)
                            nc.vector.tensor_scalar_mul(
                                vsz[:ksz, kc, po : po + 64],
                                v[:ksz, kc, h_s * DH : (h_s + 1) * DH],
                                rd[:ksz, kc : kc + 1])
                    if h_c is not None and h_c % 2 == 1:
                        nc.vector.tensor_copy(
                            ctxT[:, h_c // 2, :], box["psc"][:, 0:S])
                    pop_fill()
                # drain any leftover fillers
                while fillers:
                    fillers.popleft()()

            # ---------------- software-pipelined item loop.
            # Each item's QK projection is split: the chunks needed by the
            # first six head-slots run as fillers of the PREVIOUS item's
            # attention; the rest ("carry") run inside the item's own
            # attention, before their head-slot deadline.  This spreads
            # filler work evenly so even the last item's attention has
            # independent PE work while ScalarE drains the exp chain.
            FIRST_MS = [0, 1, 2, FC, FC + 1, FC + 2]
            CARRY_MS = [3, FC + 3, 4, FC + 4, 5, FC + 5]
            # Prologue: QK chunks to psf, V chunks to pss, alternating per
            # chunk so each pool's eviction hides under the other's matmuls.
            run_all(load_steps(0))
            if bc > 1:
                run_all(load_steps(1))
            qs, vs = qk_steps(0, FIRST_MS), v_steps(0, pool=pss)
            qchunks = [qs[i : i + DC] for i in range(0, len(qs), DC)]
            vchunks = [vs[i : i + DC] for i in range(0, len(vs), DC)]
            order = []
            for i in range(max(len(qchunks), len(vchunks))):
                if i < len(qchunks):
                    order.extend(qchunks[i])
                if i < len(vchunks):
                    order.extend(vchunks[i])
            run_all(order)
            for b in range(bc):
                fillers = deque()
                fillers.extend(qk_steps(b, CARRY_MS))  # deadline: slot 2m
                if b + 2 < bc:
                    fillers.extend(load_steps(b + 2))
                if b >= 1:
                    fillers.extend(o_steps(b - 1))
                if b + 1 < bc:
                    fillers.extend(qk_steps(b + 1, FIRST_MS))
                    fillers.extend(v_steps(b + 1))
                emit_attention(b, fillers)
            run_all(o_steps(bc - 1))

    return nc


# ---------------------------------------------------------------- host prep
def _prep_shared(Wq, bq, Wk, bk, Wv, bv, Wo, bo):
    """Build the per-core-identical weight operands."""
    scale = np.float32(1.0 / np.sqrt(DH))
    wqf = (Wq.astype(np.float32) * scale).transpose(1, 0, 2).reshape(D, D)
    wkf = Wk.astype(np.float32).transpose(1, 0, 2).reshape(D, D)
    wvf = Wv.astype(np.float32).transpose(1, 0, 2).reshape(D, D)

    def chunk4(wf):  # [d, f] -> [di, m, dc, fi]
        return wf.reshape(DC, 128, FC, 128).transpose(1, 2, 0, 3)

    wqk = np.concatenate([chunk4(wqf), chunk4(wkf)], axis=1)  # [128, 12, 6, 128]
    wv3 = wvf.reshape(DC, 128, D).transpose(1, 0, 2)          # [128, 6, 768]
    wo3 = Wo.astype(np.float32).reshape(FC, 128, D).transpose(1, 0, 2)

    bqf = (bq.astype(np.float32) * scale).reshape(D)
    bkf = bk.astype(np.float32).reshape(D)
    bqk = np.concatenate(
        [bqf.reshape(FC, 128), bkf.reshape(FC, 128)], axis=0
    ).T.copy()                                                # [128, 12]
    bvbc = np.broadcast_to(bv.astype(np.float32).reshape(D), (128, D)).copy()
    bobc = np.broadcast_to(bo.astype(np.float32).reshape(D), (128, D)).copy()

    return {
        "wqk": np.ascontiguousarray(wqk).astype(nbf),
        "wv": np.ascontiguousarray(wv3).astype(nbf),
        "wo": np.ascontiguousarray(wo3).astype(nbf),
        "bqk": np.ascontiguousarray(bqk),
        "bvbc": bvbc,
        "bobc": bobc,
    }


_NC_CACHE = {}


def kernel(x, Wq, bq, Wk, bk, Wv, bv, Wo, bo):
    x = np.asarray(x, dtype=np.float32)
    shared = _prep_shared(
        np.asarray(Wq), np.asarray(bq), np.asarray(Wk), np.asarray(bk),
        np.asarray(Wv), np.asarray(bv), np.asarray(Wo), np.asarray(bo))

    in_maps = []
    for c in range(NCORES):
        xc = x[c * BC : (c + 1) * BC]                    # [BC, S, D]
        xt = xc.transpose(2, 0, 1)                       # [D, BC, S]
        xt = xt.reshape(DC, 128, BC, S).astype(nbf)
        m = dict(shared)
        m["xt"] = np.ascontiguousarray(xt)
        in_maps.append(m)

    if "nc" not in _NC_CACHE:
        _NC_CACHE["nc"] = build_bass()
    nc = _NC_CACHE["nc"]

    res = run_bass_kernel_spmd(nc, in_maps, core_ids=list(range(NCORES)))
    out = np.concatenate([res.results[c]["out"] for c in range(NCORES)], axis=0)
    return out.astype(np.float32)


if __name__ == "__main__":
    rng = np.random.default_rng(0)
    ins = {
        "x": rng.standard_normal((B, S, D), dtype=np.float32),
        "Wq": rng.standard_normal((H, D, DH), dtype=np.float32) * 0.02,
        "bq": np.zeros((H, DH), np.float32),
        "Wk": rng.standard_normal((H, D, DH), dtype=np.float32) * 0.02,
        "bk": np.zeros((H, DH), np.float32),
        "Wv": rng.standard_normal((H, D, DH), dtype=np.float32) * 0.02,
        "bv": np.zeros((H, DH), np.float32),
        "Wo": rng.standard_normal((D, D), dtype=np.float32) * 0.02,
        "bo": np.zeros((D,), np.float32),
    }
    o = kernel(**ins)
    print("out", o.shape, o.dtype, float(np.abs(o).max()))


# revision 14
# speedup vs baseline: 1.3214x; 1.3058x over previous
"""Trainium2 Bass kernel for nn_MultiHeadAttention_31542239822105.

Math (faithful to reference, incl. softmax over the QUERY axis):
  q = einsum('bsd,hde->bhse', x, Wq) + bq ; same k, v
  scores = q @ k^T * 1/sqrt(DH)          [B,H,Sq,Sk]
  probs  = softmax(scores, axis=2)       # over q (query axis!)
  ctx    = einsum('bhqk,bhke->bhqe', probs, v)
  out    = ctx.reshape(B,S,D) @ Wo + bo

Sharding: data-parallel over batch, 8 cores x 8 batch items. No collectives.

Per-core layout strategy (all matmul contraction dims land on partitions):
  - x is pre-transposed on the HOST to xT [D, tokens] so no on-chip transposes.
  - Q^T,K^T come out of the projection f-major ([feat, token]) with W as the
    stationary operand; V comes out token-major with xT as stationary.
  - scoresT[k,q] = K^T.T @ Q^T per head -> softmax over q is a FREE-axis
    reduction; exp+sum fused into the PSUM eviction on ScalarE (accum_out).
  - 1/denominator is folded into V rows (cheap: S*DH vs S*S elements).
  - ctxT[f,q] accumulates per head pair into one PSUM tile; output projection
    uses ctxT chunks as stationary -> token-major result, direct DMA out.
  - 1/sqrt(DH) folded into Wq/bq on the host.

Schedule (the perf-critical part): the ScalarE exp/accum chain (~830ns per
128x577 tile) is slower than the PE's scores+ctx work for the same tile
(~480ns), so a per-item serial stage order starves the PE during attention.
Instead the emission order software-pipelines ACROSS batch items: while
item b's attention drains on ScalarE, the PE executes interleaved "filler"
matmuls from item b+1's QK/V projections and item b-1's output projection.
Fillers are popped from a queue between the scores/ctx matmul pairs, so the
in-order PE stream always has independent work and stays in its fast clock
state (p-state ramps to 2.4 GHz only after ~3us of continuous execution).

Engine balance per item: PE ~65us, ScalarE(exp+accum) ~50us, DVE (all
PSUM evictions) ~15us, GpSimd (1/den folding into V) ~10us.

K^T and the scaled-V operands are zero-PADDED per head so the scores/ctx
lhsT is a full 128-partition operand (half-shape matmuls drop the PE out of
its fast clock); the zero halves live in persistent double-buffered tiles
that are memset once at kernel start, never per item.
"""

import sys
from collections import deque

if "/opt/trn_rl_repo" not in sys.path:
    sys.path.insert(0, "/opt/trn_rl_repo")

import numpy as np
import ml_dtypes

import concourse.bass as bass
import concourse.mybir as mybir
import concourse.tile as tile_mod
from concourse.vector_clock import ScopedClock
from concourse.bass_utils import run_bass_kernel_spmd

# ---------------------------------------------------------------- constants
B, S, D, H = 64, 577, 768, 12
DH = D // H          # 64
NCORES = 8
BC = B // NCORES     # 8 batch items per core
DC = D // 128        # 6 d-chunks
FC = D // 128        # 6 f-chunks per projection matrix
M_QK = 2 * FC        # 12 combined Q+K f-chunks
TT = (S + 127) // 128  # 5 token tiles (128,128,128,128,65)
S0 = 512             # PSUM-bank-sized free-dim split: 577 = 512 + 65
S1 = S - S0

BF16 = mybir.dt.bfloat16
F32 = mybir.dt.float32
nbf = ml_dtypes.bfloat16

_TILE_PATCHED = False
_CUR_NC = [None]


def _patch_tile_drain():
    """The walrus build here rejects >1 sync-wait per instruction
    ("Too many sync wait commands"). Two patches:
    1. post-legalize pass that moves extra waits onto single-wait nops
       inserted just before the offending instruction (same engine);
    2. the final SP Drain (emitted after legalize) gets the same split.
    """
    global _TILE_PATCHED
    if _TILE_PATCHED:
        return
    _TILE_PATCHED = True

    _orig_postorder = tile_mod.postorder_instruction_blocks

    def _split_multi_waits(ordered, nc):
        for bbname, insts in ordered.items():
            out = []
            n_split = 0
            for inst in insts:
                si = inst.sync_info
                if si is not None and len(si.on_wait) > 1:
                    waits = list(si.on_wait)
                    for w in waits[:-1]:
                        nop = mybir.InstNoOp(
                            name=nc.get_next_instruction_name(),
                            ins=[],
                            outs=[],
                            bass_is_fusable=False,
                        )
                        nop.engine = inst.engine
                        nop.sync_info = mybir.SyncInfo(on_wait=[w], on_update=[])
                        nc.register_instruction(nop, overwrite=True)
                        out.append(nop)
                        n_split += 1
                    inst.sync_info = mybir.SyncInfo(
                        on_wait=[waits[-1]], on_update=list(si.on_update)
                    )
                out.append(inst)
            ordered[bbname] = out
        return ordered

    def postorder_and_split(ordered, start_bb, postordered):
        # Runs post-sem-assignment, right before lowering: the only spot
        # where the final per-instruction waits are visible and editable.
        nc = _CUR_NC[0]
        _split_multi_waits(ordered, nc)
        return _orig_postorder(ordered, start_bb, postordered)

    tile_mod.postorder_instruction_blocks = postorder_and_split

    def _drain_and_barrier_split(self, tick_clock, wait_clock):
        nc = self.nc
        drain_inst = nc.sync.drain()
        wait_clock.add_sem_waits(
            drain_inst.ins, ScopedClock({None: tick_clock.global_clock})
        )
        si = drain_inst.ins.sync_info
        waits = list(si.on_wait)
        if len(waits) > 1:
            drain_inst.ins.sync_info = mybir.SyncInfo(
                on_wait=[waits[0]], on_update=list(si.on_update)
            )
            for w in waits[1:]:
                nop = nc.sync.nop(nofuse=True)
                nop.ins.sync_info = mybir.SyncInfo(on_wait=[w], on_update=[])
        nc.all_engine_barrier()
        assert self.sems is not None
        popped = nc._tile_sem_poison_stack.pop()
        assert popped is self._sem_poison
        nc.clear_and_free_semaphores(list(self.sems.allocated().values()))
        nc.all_engine_barrier()

    tile_mod.TileContext._drain_and_barrier = _drain_and_barrier_split


# ---------------------------------------------------------------- builder
def build_bass(bc=BC):
    """Emit the per-core kernel for `bc` batch items. Returns nc."""
    _patch_tile_drain()
    nc = bass.Bass()
    _CUR_NC[0] = nc

    xt_d = nc.declare_dram_parameter("xt", [DC, 128, bc, S], BF16, isOutput=False)
    wqk_d = nc.declare_dram_parameter("wqk", [128, M_QK, DC, 128], BF16, isOutput=False)
    wv_d = nc.declare_dram_parameter("wv", [128, DC, D], BF16, isOutput=False)
    wo_d = nc.declare_dram_parameter("wo", [128, FC, D], BF16, isOutput=False)
    bqk_d = nc.declare_dram_parameter("bqk", [128, M_QK], F32, isOutput=False)
    bvbc_d = nc.declare_dram_parameter("bvbc", [128, D], F32, isOutput=False)
    bobc_d = nc.declare_dram_parameter("bobc", [128, D], F32, isOutput=False)
    out_d = nc.declare_dram_parameter("out", [bc, S, D], F32, isOutput=True)

    AF = mybir.ActivationFunctionType

    with tile_mod.TileContext(nc) as tc:
        with (
            tc.tile_pool(name="singles", bufs=1) as singles,
            tc.tile_pool(name="xt", bufs=3) as xpool,
            tc.tile_pool(name="qk", bufs=2) as qkpool,
            tc.tile_pool(name="v", bufs=2) as vpool,
            tc.tile_pool(name="probs", bufs=4) as ppool,
            tc.tile_pool(name="den", bufs=3) as dpool,
            tc.tile_pool(name="ctx", bufs=2) as cpool,
            tc.tile_pool(name="ot", bufs=3) as opool,
            tc.tile_pool(name="pss", bufs=2, space="PSUM") as pss,
            tc.tile_pool(name="psc", bufs=1, space="PSUM") as psc,
            tc.tile_pool(name="psf", bufs=1, space="PSUM") as psf,
        ):
            # -------- resident weights / biases.  wqk is split per d-chunk so
            # the first projection matmul only waits on its own slice.
            wqk = singles.tile([128, M_QK, DC, 128], BF16, tag="wqk")
            for dc in range(DC):
                nc.sync.dma_start(out=wqk[:, :, dc, :], in_=wqk_d[:, :, dc, :])
            bqk = singles.tile([128, M_QK], F32, tag="bqk")
            nc.sync.dma_start(out=bqk, in_=bqk_d[:])
            wv = singles.tile([128, DC, D], BF16, tag="wv")
            nc.sync.dma_start(out=wv, in_=wv_d[:])
            wo = singles.tile([128, FC, D], BF16, tag="wo")
            nc.sync.dma_start(out=wo, in_=wo_d[:])
            bvbc = singles.tile([128, D], F32, tag="bvbc")
            nc.sync.dma_start(out=bvbc, in_=bvbc_d[:])
            bobc = singles.tile([128, D], F32, tag="bobc")
            nc.sync.dma_start(out=bobc, in_=bobc_d[:])

            # Persistent zero-padded operand buffers (double-buffered by item
            # parity for ktz, by head parity for vsz).  The zero halves are
            # written ONCE here and never touched again.
            ktzs = [
                singles.tile([128, FC, 2, S], BF16, tag=f"ktz{i}", name=f"ktz{i}")
                for i in range(2)
            ]
            vszs = [
                singles.tile([128, TT, 128], BF16, tag=f"vsz{i}", name=f"vsz{i}")
                for i in range(2)
            ]
            for i in range(2):
                for mk in range(FC):
                    nc.vector.memset(ktzs[i][64:128, mk, 0, :], 0.0)
                    nc.vector.memset(ktzs[i][0:64, mk, 1, :], 0.0)
            nc.vector.memset(vszs[0][:, :, 64:128], 0.0)
            nc.vector.memset(vszs[1][:, :, 0:64], 0.0)

            st = {}  # cross-stage tile handles: ('xt'|'qk'|'v'|'ctx', b)

            # ---------------- filler-step generators (one PE matmul pair
            # or one DMA batch per step; evictions ride along).
            # Filler steps are (emit_fn, chunk_start) pairs.  chunk_start
            # marks the first matmul of a PSUM accumulation chunk: the
            # interleaver never lets it follow the previous chunk's last
            # step inside one pop batch, so the single-buffered psf pool's
            # eviction latency always hides under a scores/ctx pair.
            def load_steps(b):
                def go():
                    xt = xpool.tile([128, DC, S], BF16, tag="xt", name="xt")
                    st[("xt", b)] = xt
                    for dc in range(DC):
                        nc.sync.dma_start(out=xt[:, dc, :], in_=xt_d[dc, :, b, :])

                return [(go, False)]

            def qk_steps(b, ms=None):
                steps = []
                box = {}
                for m in ms if ms is not None else range(M_QK):
                    for dc in range(DC):
                        def go(m=m, dc=dc):
                            xt = st[("xt", b)]
                            if ("qk", b) not in st:
                                st[("qk", b)] = qkpool.tile(
                                    [128, FC, S], BF16, tag="qk", name="qk"
                                )
                            if dc == 0:
                                box["ps"] = psf.tile(
                                    [128, D], F32, tag="psf", name="psfq"
                                )
                            ps = box["ps"]
                            stt, spp = dc == 0, dc == DC - 1
                            nc.tensor.matmul(
                                ps[:, 0:S0], lhsT=wqk[:, m, dc, :],
                                rhs=xt[:, dc, 0:S0], start=stt, stop=spp)
                            nc.tensor.matmul(
                                ps[:, S0:S], lhsT=wqk[:, m, dc, :],
                                rhs=xt[:, dc, S0:S], start=stt, stop=spp)
                            if spp:
                                if m < FC:
                                    nc.vector.tensor_scalar_add(
                                        st[("qk", b)][:, m, :], ps[:, 0:S],
                                        bqk[:, m : m + 1])
                                else:
                                    mk = m - FC
                                    ktz = ktzs[b % 2]
                                    nc.vector.tensor_scalar_add(
                                        ktz[0:64, mk, 0, :], ps[0:64, 0:S],
                                        bqk[0:64, m : m + 1])
                                    nc.vector.tensor_scalar_add(
                                        ktz[64:128, mk, 1, :], ps[64:128, 0:S],
                                        bqk[64:128, m : m + 1])

                        steps.append((go, dc == 0))
                return steps

            def v_steps(b, pool=None):
                steps = []
                box = {}
                for tt in range(TT):
                    for dc in range(DC):
                        def go(tt=tt, dc=dc):
                            xt = st[("xt", b)]
                            tsz = min(128, S - tt * 128)
                            t0 = tt * 128
                            if tt == 0 and dc == 0:
                                st[("v", b)] = vpool.tile(
                                    [128, TT, D], BF16, tag="v", name="v"
                                )
                            if dc == 0:
                                p = pool if pool is not None else psf
                                tg = "pss" if pool is not None else "psf"
                                box["ps"] = p.tile(
                                    [128, D], F32, tag=tg, name="psfv"
                                )
                            ps = box["ps"]
                            stt, spp = dc == 0, dc == DC - 1
                            nc.tensor.matmul(
                                ps[:tsz, 0:S0], lhsT=xt[:, dc, t0 : t0 + tsz],
                                rhs=wv[:, dc, 0:S0], start=stt, stop=spp)
                            nc.tensor.matmul(
                                ps[:tsz, S0:D], lhsT=xt[:, dc, t0 : t0 + tsz],
                                rhs=wv[:, dc, S0:D], start=stt, stop=spp)
                            if spp:
                                nc.vector.tensor_add(
                                    st[("v", b)][:tsz, tt, :], ps[:tsz, 0:D],
                                    bvbc[:tsz])

                        steps.append((go, dc == 0))
                return steps

            def o_steps(b):
                steps = []
                box = {}
                for tt in range(TT):
                    for fc in range(FC):
                        def go(tt=tt, fc=fc):
                            ctxT = st[("ctx", b)]
                            tsz = min(128, S - tt * 128)
                            t0 = tt * 128
                            if fc == 0:
                                box["ps"] = psf.tile(
                                    [128, D], F32, tag="psf", name="psfo"
                                )
                            ps = box["ps"]
                            stt, spp = fc == 0, fc == FC - 1
                            nc.tensor.matmul(
                                ps[:tsz, 0:S0], lhsT=ctxT[:, fc, t0 : t0 + tsz],
                                rhs=wo[:, fc, 0:S0], start=stt, stop=spp)
                            nc.tensor.matmul(
                                ps[:tsz, S0:D], lhsT=ctxT[:, fc, t0 : t0 + tsz],
                                rhs=wo[:, fc, S0:D], start=stt, stop=spp)
                            if spp:
                                ot = opool.tile([128, D], F32, tag="ot", name="ot")
                                nc.vector.tensor_add(
                                    ot[:tsz], ps[:tsz, 0:D], bobc[:tsz])
                                nc.sync.dma_start(
                                    out=out_d[b, t0 : t0 + tsz, :], in_=ot[:tsz])

                        steps.append((go, fc == 0))
                return steps

            def run_all(steps):
                for s, _ in steps:
                    s()

            # ---------------- attention for item b, fillers interleaved
            def emit_attention(b, fillers):
                qk = st[("qk", b)]
                v = st[("v", b)]
                ktz = ktzs[b % 2]
                ctxT = cpool.tile([128, FC, S], BF16, tag="ctx", name="ctx")
                st[("ctx", b)] = ctxT

                nslots = H + 2
                # pop points: one after each kc iteration + one per slot end
                points = {"left": nslots * (TT + 1)}

                def pop_fill():
                    points["left"] -= 1
                    if not fillers:
                        return
                    n = (len(fillers) + max(points["left"], 1) - 1) // max(
                        points["left"], 1
                    )
                    popped = 0
                    while fillers and popped < n:
                        fn, chunk_start = fillers[0]
                        if chunk_start and popped > 0:
                            break  # new chunk starts at the NEXT pop point
                        fillers.popleft()
                        fn()
                        popped += 1

                hstate = {}  # h -> probs tile
                box = {"psc": None}
                for hs in range(nslots):
                    h_s = hs if hs < H else None
                    h_c = hs - 2 if hs >= 2 else None
                    if h_s is not None:
                        m, j = h_s // 2, h_s % 2
                        probs = ppool.tile(
                            [128, TT, S], BF16, tag="probs", name="probs"
                        )
                        den = dpool.tile([128, TT], F32, tag="den", name="den")
                        nc.vector.memset(den, 1.0)
                        hstate[h_s] = probs
                    if h_c is not None:
                        probs_c = hstate.pop(h_c) if h_c % 2 == 1 else hstate[h_c]
                        if h_c % 2 == 0:
                            box["psc"] = psc.tile(
                                [128, D], F32, tag="psc", name="pscx"
                            )
                        vsz_c = vszs[h_c % 2]
                    for kc in range(TT):
                        ksz = min(128, S - kc * 128)
                        k0 = kc * 128
                        if h_s is not None:
                            ps = pss.tile([128, D], F32, tag="pss", name="pssc")
                            nc.tensor.matmul(
                                ps[:ksz, 0:S0],
                                lhsT=ktz[:, m, j, k0 : k0 + ksz],
                                rhs=qk[:, m, 0:S0], start=True, stop=True)
                            nc.tensor.matmul(
                                ps[:ksz, S0:S],
                                lhsT=ktz[:, m, j, k0 : k0 + ksz],
                                rhs=qk[:, m, S0:S], start=True, stop=True)
                            # exp + row-sum (over q) fused in the eviction
                            nc.scalar.activation(
                                probs[:ksz, kc, :], ps[:ksz, 0:S], AF.Exp,
                                accum_out=den[:ksz, kc : kc + 1])
                        if h_c is not None:
                            stt = (h_c % 2 == 0) and kc == 0
                            spp = (h_c % 2 == 1) and kc == TT - 1
                            nc.tensor.matmul(
                                box["psc"][:, 0:S0], lhsT=vsz_c[:ksz, kc, :],
                                rhs=probs_c[:ksz, kc, 0:S0], start=stt, stop=spp)
                            nc.tensor.matmul(
                                box["psc"][:, S0:S], lhsT=vsz_c[:ksz, kc, :],
                                rhs=probs_c[:ksz, kc, S0:S], start=stt, stop=spp)
                        pop_fill()
                    if h_s is not None:
                        # fold 1/denominator into this head's V rows (GpSimd;
                        # writes only the live half of the persistent buffer)
                        po = j * 64
                        rd = dpool.tile([128, TT], F32, tag="rd", name="rd")
                        nc.vector.reciprocal(rd, den)
                        vsz = vszs[j]
                        for kc in range(TT):
                            ksz = min(128, S - kc * 128)
                            nc.vector.tensor_scalar_mul(
                                vsz[:ksz, kc, po : po + 64],
                                v[:ksz, kc, h_s * DH : (h_s + 1) * DH],
                                rd[:ksz, kc : kc + 1])
                    if h_c is not None and h_c % 2 == 1:
                        nc.vector.tensor_copy(
                            ctxT[:, h_c // 2, :], box["psc"][:, 0:S])
                    pop_fill()
                # drain any leftover fillers
                while fillers:
                    fillers.popleft()[0]()

            # ---------------- software-pipelined item loop.
            # Each item's QK projection is split: the chunks needed by the
            # first six head-slots run as fillers of the PREVIOUS item's
            # attention; the rest ("carry") run inside the item's own
            # attention, before their head-slot deadline.  This spreads
            # filler work evenly so even the last item's attention has
            # independent PE work while ScalarE drains the exp chain.
            FIRST_MS = [0, 1, 2, FC, FC + 1, FC + 2]
            CARRY_MS = [3, FC + 3, 4, FC + 4, 5, FC + 5]
            # Prologue: QK chunks to psf, V chunks to pss, alternating per
            # chunk so each pool's eviction hides under the other's matmuls.
            run_all(load_steps(0))
            if bc > 1:
                run_all(load_steps(1))
            qs, vs = qk_steps(0, FIRST_MS), v_steps(0, pool=pss)
            qchunks = [qs[i : i + DC] for i in range(0, len(qs), DC)]
            vchunks = [vs[i : i + DC] for i in range(0, len(vs), DC)]
            order = []
            for i in range(max(len(qchunks), len(vchunks))):
                if i < len(qchunks):
                    order.extend(qchunks[i])
                if i < len(vchunks):
                    order.extend(vchunks[i])
            run_all(order)
            for b in range(bc):
                fillers = deque()
                fillers.extend(qk_steps(b, CARRY_MS))  # deadline: slot 2m
                if b + 2 < bc:
                    fillers.extend(load_steps(b + 2))
                if b >= 1:
                    fillers.extend(o_steps(b - 1))
                if b + 1 < bc:
                    fillers.extend(qk_steps(b + 1, FIRST_MS))
                    fillers.extend(v_steps(b + 1))
                emit_attention(b, fillers)
            run_all(o_steps(bc - 1))

    return nc


# ---------------------------------------------------------------- host prep
def _prep_shared(Wq, bq, Wk, bk, Wv, bv, Wo, bo):
    """Build the per-core-identical weight operands."""
    scale = np.float32(1.0 / np.sqrt(DH))
    wqf = (Wq.astype(np.float32) * scale).transpose(1, 0, 2).reshape(D, D)
    wkf = Wk.astype(np.float32).transpose(1, 0, 2).reshape(D, D)
    wvf = Wv.astype(np.float32).transpose(1, 0, 2).reshape(D, D)

    def chunk4(wf):  # [d, f] -> [di, m, dc, fi]
        return wf.reshape(DC, 128, FC, 128).transpose(1, 2, 0, 3)

    wqk = np.concatenate([chunk4(wqf), chunk4(wkf)], axis=1)  # [128, 12, 6, 128]
    wv3 = wvf.reshape(DC, 128, D).transpose(1, 0, 2)          # [128, 6, 768]
    wo3 = Wo.astype(np.float32).reshape(FC, 128, D).transpose(1, 0, 2)

    bqf = (bq.astype(np.float32) * scale).reshape(D)
    bkf = bk.astype(np.float32).reshape(D)
    bqk = np.concatenate(
        [bqf.reshape(FC, 128), bkf.reshape(FC, 128)], axis=0
    ).T.copy()                                                # [128, 12]
    bvbc = np.broadcast_to(bv.astype(np.float32).reshape(D), (128, D)).copy()
    bobc = np.broadcast_to(bo.astype(np.float32).reshape(D), (128, D)).copy()

    return {
        "wqk": np.ascontiguousarray(wqk).astype(nbf),
        "wv": np.ascontiguousarray(wv3).astype(nbf),
        "wo": np.ascontiguousarray(wo3).astype(nbf),
        "bqk": np.ascontiguousarray(bqk),
        "bvbc": bvbc,
        "bobc": bobc,
    }


_NC_CACHE = {}


def kernel(x, Wq, bq, Wk, bk, Wv, bv, Wo, bo):
    x = np.asarray(x, dtype=np.float32)
    shared = _prep_shared(
        np.asarray(Wq), np.asarray(bq), np.asarray(Wk), np.asarray(bk),
        np.asarray(Wv), np.asarray(bv), np.asarray(Wo), np.asarray(bo))

    in_maps = []
    for c in range(NCORES):
        xc = x[c * BC : (c + 1) * BC]                    # [BC, S, D]
        xt = xc.transpose(2, 0, 1)                       # [D, BC, S]
        xt = xt.reshape(DC, 128, BC, S).astype(nbf)
        m = dict(shared)
        m["xt"] = np.ascontiguousarray(xt)
        in_maps.append(m)

    if "nc" not in _NC_CACHE:
        _NC_CACHE["nc"] = build_bass()
    nc = _NC_CACHE["nc"]

    res = run_bass_kernel_spmd(nc, in_maps, core_ids=list(range(NCORES)))
    out = np.concatenate([res.results[c]["out"] for c in range(NCORES)], axis=0)
    return out.astype(np.float32)


if __name__ == "__main__":
    rng = np.random.default_rng(0)
    ins = {
        "x": rng.standard_normal((B, S, D), dtype=np.float32),
        "Wq": rng.standard_normal((H, D, DH), dtype=np.float32) * 0.02,
        "bq": np.zeros((H, DH), np.float32),
        "Wk": rng.standard_normal((H, D, DH), dtype=np.float32) * 0.02,
        "bk": np.zeros((H, DH), np.float32),
        "Wv": rng.standard_normal((H, D, DH), dtype=np.float32) * 0.02,
        "bv": np.zeros((H, DH), np.float32),
        "Wo": rng.standard_normal((D, D), dtype=np.float32) * 0.02,
        "bo": np.zeros((D,), np.float32),
    }
    o = kernel(**ins)
    print("out", o.shape, o.dtype, float(np.abs(o).max()))


# revision 15
# speedup vs baseline: 1.3475x; 1.0198x over previous
"""Trainium2 Bass kernel for nn_MultiHeadAttention_31542239822105.

Math (faithful to reference, incl. softmax over the QUERY axis):
  q = einsum('bsd,hde->bhse', x, Wq) + bq ; same k, v
  scores = q @ k^T * 1/sqrt(DH)          [B,H,Sq,Sk]
  probs  = softmax(scores, axis=2)       # over q (query axis!)
  ctx    = einsum('bhqk,bhke->bhqe', probs, v)
  out    = ctx.reshape(B,S,D) @ Wo + bo

Sharding: data-parallel over batch, 8 cores x 8 batch items. No collectives.

Per-core layout strategy (all matmul contraction dims land on partitions):
  - x is pre-transposed on the HOST to xT [D, tokens] so no on-chip transposes.
  - Q^T,K^T come out of the projection f-major ([feat, token]) with W as the
    stationary operand; V comes out token-major with xT as stationary.
  - scoresT[k,q] = K^T.T @ Q^T per head -> softmax over q is a FREE-axis
    reduction; exp+sum fused into the PSUM eviction on ScalarE (accum_out).
  - 1/denominator is folded into V rows (cheap: S*DH vs S*S elements).
  - ctxT[f,q] accumulates per head pair into one PSUM tile; output projection
    uses ctxT chunks as stationary -> token-major result, direct DMA out.
  - 1/sqrt(DH) folded into Wq/bq on the host.

Schedule (the perf-critical part): the ScalarE exp/accum chain (~830ns per
128x577 tile) is slower than the PE's scores+ctx work for the same tile
(~480ns), so a per-item serial stage order starves the PE during attention.
Instead the emission order software-pipelines ACROSS batch items: while
item b's attention drains on ScalarE, the PE executes interleaved "filler"
matmuls from item b+1's QK/V projections and item b-1's output projection.
Fillers are popped from a queue between the scores/ctx matmul pairs, so the
in-order PE stream always has independent work and stays in its fast clock
state (p-state ramps to 2.4 GHz only after ~3us of continuous execution).

Engine balance per item: PE ~65us, ScalarE(exp+accum) ~50us, DVE (all
PSUM evictions) ~15us, GpSimd (1/den folding into V) ~10us.

K^T and the scaled-V operands are zero-PADDED per head so the scores/ctx
lhsT is a full 128-partition operand (half-shape matmuls drop the PE out of
its fast clock); the zero halves live in persistent double-buffered tiles
that are memset once at kernel start, never per item.
"""

import sys
from collections import deque

if "/opt/trn_rl_repo" not in sys.path:
    sys.path.insert(0, "/opt/trn_rl_repo")

import numpy as np
import ml_dtypes

import concourse.bass as bass
import concourse.mybir as mybir
import concourse.tile as tile_mod
from concourse.vector_clock import ScopedClock
from concourse.bass_utils import run_bass_kernel_spmd

# ---------------------------------------------------------------- constants
B, S, D, H = 64, 577, 768, 12
DH = D // H          # 64
NCORES = 8
BC = B // NCORES     # 8 batch items per core
DC = D // 128        # 6 d-chunks
FC = D // 128        # 6 f-chunks per projection matrix
M_QK = 2 * FC        # 12 combined Q+K f-chunks
TT = (S + 127) // 128  # 5 token tiles (128,128,128,128,65)
S0 = 512             # PSUM-bank-sized free-dim split: 577 = 512 + 65
S1 = S - S0

BF16 = mybir.dt.bfloat16
F32 = mybir.dt.float32
nbf = ml_dtypes.bfloat16

_TILE_PATCHED = False
_CUR_NC = [None]


def _patch_tile_drain():
    """The walrus build here rejects >1 sync-wait per instruction
    ("Too many sync wait commands"). Two patches:
    1. post-legalize pass that moves extra waits onto single-wait nops
       inserted just before the offending instruction (same engine);
    2. the final SP Drain (emitted after legalize) gets the same split.
    """
    global _TILE_PATCHED
    if _TILE_PATCHED:
        return
    _TILE_PATCHED = True

    _orig_postorder = tile_mod.postorder_instruction_blocks

    def _split_multi_waits(ordered, nc):
        for bbname, insts in ordered.items():
            out = []
            n_split = 0
            for inst in insts:
                si = inst.sync_info
                if si is not None and len(si.on_wait) > 1:
                    waits = list(si.on_wait)
                    for w in waits[:-1]:
                        nop = mybir.InstNoOp(
                            name=nc.get_next_instruction_name(),
                            ins=[],
                            outs=[],
                            bass_is_fusable=False,
                        )
                        nop.engine = inst.engine
                        nop.sync_info = mybir.SyncInfo(on_wait=[w], on_update=[])
                        nc.register_instruction(nop, overwrite=True)
                        out.append(nop)
                        n_split += 1
                    inst.sync_info = mybir.SyncInfo(
                        on_wait=[waits[-1]], on_update=list(si.on_update)
                    )
                out.append(inst)
            ordered[bbname] = out
        return ordered

    def postorder_and_split(ordered, start_bb, postordered):
        # Runs post-sem-assignment, right before lowering: the only spot
        # where the final per-instruction waits are visible and editable.
        nc = _CUR_NC[0]
        _split_multi_waits(ordered, nc)
        return _orig_postorder(ordered, start_bb, postordered)

    tile_mod.postorder_instruction_blocks = postorder_and_split

    def _drain_and_barrier_split(self, tick_clock, wait_clock):
        nc = self.nc
        drain_inst = nc.sync.drain()
        wait_clock.add_sem_waits(
            drain_inst.ins, ScopedClock({None: tick_clock.global_clock})
        )
        si = drain_inst.ins.sync_info
        waits = list(si.on_wait)
        if len(waits) > 1:
            drain_inst.ins.sync_info = mybir.SyncInfo(
                on_wait=[waits[0]], on_update=list(si.on_update)
            )
            for w in waits[1:]:
                nop = nc.sync.nop(nofuse=True)
                nop.ins.sync_info = mybir.SyncInfo(on_wait=[w], on_update=[])
        nc.all_engine_barrier()
        assert self.sems is not None
        popped = nc._tile_sem_poison_stack.pop()
        assert popped is self._sem_poison
        nc.clear_and_free_semaphores(list(self.sems.allocated().values()))
        nc.all_engine_barrier()

    tile_mod.TileContext._drain_and_barrier = _drain_and_barrier_split


# ---------------------------------------------------------------- builder
def build_bass(bc=BC):
    """Emit the per-core kernel for `bc` batch items. Returns nc."""
    _patch_tile_drain()
    nc = bass.Bass()
    _CUR_NC[0] = nc

    xt_d = nc.declare_dram_parameter("xt", [DC, 128, bc, S], BF16, isOutput=False)
    wqk_d = nc.declare_dram_parameter("wqk", [DC, 128, M_QK, 128], BF16, isOutput=False)
    wv_d = nc.declare_dram_parameter("wv", [128, DC, D], BF16, isOutput=False)
    wo_d = nc.declare_dram_parameter("wo", [128, FC, D], BF16, isOutput=False)
    bqk_d = nc.declare_dram_parameter("bqk", [128, M_QK], F32, isOutput=False)
    bvbc_d = nc.declare_dram_parameter("bvbc", [128, D], F32, isOutput=False)
    bobc_d = nc.declare_dram_parameter("bobc", [128, D], F32, isOutput=False)
    out_d = nc.declare_dram_parameter("out", [bc, S, D], F32, isOutput=True)

    AF = mybir.ActivationFunctionType

    with tile_mod.TileContext(nc) as tc:
        with (
            tc.tile_pool(name="singles", bufs=1) as singles,
            tc.tile_pool(name="xt", bufs=3) as xpool,
            tc.tile_pool(name="qk", bufs=2) as qkpool,
            tc.tile_pool(name="v", bufs=2) as vpool,
            tc.tile_pool(name="probs", bufs=4) as ppool,
            tc.tile_pool(name="den", bufs=3) as dpool,
            tc.tile_pool(name="ctx", bufs=2) as cpool,
            tc.tile_pool(name="ot", bufs=3) as opool,
            tc.tile_pool(name="pss", bufs=2, space="PSUM") as pss,
            tc.tile_pool(name="psc", bufs=1, space="PSUM") as psc,
            tc.tile_pool(name="psf", bufs=1, space="PSUM") as psf,
        ):
            # -------- resident weights / biases.  wqk is split per d-chunk so
            # the first projection matmul only waits on its own slice.
            wqk = singles.tile([128, DC, M_QK, 128], BF16, tag="wqk")
            for dc in range(DC):
                nc.sync.dma_start(out=wqk[:, dc, :, :], in_=wqk_d[dc, :, :, :])
            bqk = singles.tile([128, M_QK], F32, tag="bqk")
            nc.sync.dma_start(out=bqk, in_=bqk_d[:])
            wv = singles.tile([128, DC, D], BF16, tag="wv")
            nc.sync.dma_start(out=wv, in_=wv_d[:])
            wo = singles.tile([128, FC, D], BF16, tag="wo")
            nc.sync.dma_start(out=wo, in_=wo_d[:])
            bvbc = singles.tile([128, D], F32, tag="bvbc")
            nc.sync.dma_start(out=bvbc, in_=bvbc_d[:])
            bobc = singles.tile([128, D], F32, tag="bobc")
            nc.sync.dma_start(out=bobc, in_=bobc_d[:])

            # Persistent zero-padded operand buffers (double-buffered by item
            # parity for ktz, by head parity for vsz).  The zero halves are
            # written ONCE here and never touched again.
            ktzs = [
                singles.tile([128, FC, 2, S], BF16, tag=f"ktz{i}", name=f"ktz{i}")
                for i in range(2)
            ]
            vszs = [
                singles.tile([128, TT, 128], BF16, tag=f"vsz{i}", name=f"vsz{i}")
                for i in range(2)
            ]
            for i in range(2):
                for mk in range(FC):
                    nc.vector.memset(ktzs[i][64:128, mk, 0, :], 0.0)
                    nc.vector.memset(ktzs[i][0:64, mk, 1, :], 0.0)
            nc.vector.memset(vszs[0][:, :, 64:128], 0.0)
            nc.vector.memset(vszs[1][:, :, 0:64], 0.0)

            st = {}  # cross-stage tile handles: ('xt'|'qk'|'v'|'ctx', b)

            # ---------------- filler-step generators (one PE matmul pair
            # or one DMA batch per step; evictions ride along).
            # Filler steps are (emit_fn, chunk_start) pairs.  chunk_start
            # marks the first matmul of a PSUM accumulation chunk: the
            # interleaver never lets it follow the previous chunk's last
            # step inside one pop batch, so the single-buffered psf pool's
            # eviction latency always hides under a scores/ctx pair.
            def load_steps(b):
                def go():
                    xt = xpool.tile([128, DC, S], BF16, tag="xt", name="xt")
                    st[("xt", b)] = xt
                    for dc in range(DC):
                        nc.sync.dma_start(out=xt[:, dc, :], in_=xt_d[dc, :, b, :])

                return [(go, False)]

            def qk_steps(b, ms=None):
                steps = []
                box = {}
                for m in ms if ms is not None else range(M_QK):
                    for dc in range(DC):
                        def go(m=m, dc=dc):
                            xt = st[("xt", b)]
                            if ("qk", b) not in st:
                                st[("qk", b)] = qkpool.tile(
                                    [128, FC, S], BF16, tag="qk", name="qk"
                                )
                            if dc == 0:
                                box["ps"] = psf.tile(
                                    [128, D], F32, tag="psf", name="psfq"
                                )
                            ps = box["ps"]
                            stt, spp = dc == 0, dc == DC - 1
                            nc.tensor.matmul(
                                ps[:, 0:S0], lhsT=wqk[:, dc, m, :],
                                rhs=xt[:, dc, 0:S0], start=stt, stop=spp)
                            nc.tensor.matmul(
                                ps[:, S0:S], lhsT=wqk[:, dc, m, :],
                                rhs=xt[:, dc, S0:S], start=stt, stop=spp)
                            if spp:
                                if m < FC:
                                    nc.vector.tensor_scalar_add(
                                        st[("qk", b)][:, m, :], ps[:, 0:S],
                                        bqk[:, m : m + 1])
                                else:
                                    mk = m - FC
                                    ktz = ktzs[b % 2]
                                    nc.vector.tensor_scalar_add(
                                        ktz[0:64, mk, 0, :], ps[0:64, 0:S],
                                        bqk[0:64, m : m + 1])
                                    nc.vector.tensor_scalar_add(
                                        ktz[64:128, mk, 1, :], ps[64:128, 0:S],
                                        bqk[64:128, m : m + 1])

                        steps.append((go, dc == 0))
                return steps

            def v_steps(b, pool=None):
                steps = []
                box = {}
                for tt in range(TT):
                    for dc in range(DC):
                        def go(tt=tt, dc=dc):
                            xt = st[("xt", b)]
                            tsz = min(128, S - tt * 128)
                            t0 = tt * 128
                            if tt == 0 and dc == 0:
                                st[("v", b)] = vpool.tile(
                                    [128, TT, D], BF16, tag="v", name="v"
                                )
                            if dc == 0:
                                p = pool if pool is not None else psf
                                tg = "pss" if pool is not None else "psf"
                                box["ps"] = p.tile(
                                    [128, D], F32, tag=tg, name="psfv"
                                )
                            ps = box["ps"]
                            stt, spp = dc == 0, dc == DC - 1
                            nc.tensor.matmul(
                                ps[:tsz, 0:S0], lhsT=xt[:, dc, t0 : t0 + tsz],
                                rhs=wv[:, dc, 0:S0], start=stt, stop=spp)
                            nc.tensor.matmul(
                                ps[:tsz, S0:D], lhsT=xt[:, dc, t0 : t0 + tsz],
                                rhs=wv[:, dc, S0:D], start=stt, stop=spp)
                            if spp:
                                nc.vector.tensor_add(
                                    st[("v", b)][:tsz, tt, :], ps[:tsz, 0:D],
                                    bvbc[:tsz])

                        steps.append((go, dc == 0))
                return steps

            def o_steps(b, alt=False):
                steps = []
                box = {}
                for tt in range(TT):
                    for fc in range(FC):
                        def go(tt=tt, fc=fc):
                            ctxT = st[("ctx", b)]
                            tsz = min(128, S - tt * 128)
                            t0 = tt * 128
                            if fc == 0:
                                # alt: alternate chunks between psf and the
                                # (idle at epilogue) pss pool so the single
                                # psf buffer's eviction latency never stalls
                                # back-to-back chunks
                                if alt and tt % 2 == 1:
                                    box["ps"] = pss.tile(
                                        [128, D], F32, tag="pss", name="psso"
                                    )
                                else:
                                    box["ps"] = psf.tile(
                                        [128, D], F32, tag="psf", name="psfo"
                                    )
                            ps = box["ps"]
                            stt, spp = fc == 0, fc == FC - 1
                            nc.tensor.matmul(
                                ps[:tsz, 0:S0], lhsT=ctxT[:, fc, t0 : t0 + tsz],
                                rhs=wo[:, fc, 0:S0], start=stt, stop=spp)
                            nc.tensor.matmul(
                                ps[:tsz, S0:D], lhsT=ctxT[:, fc, t0 : t0 + tsz],
                                rhs=wo[:, fc, S0:D], start=stt, stop=spp)
                            if spp:
                                ot = opool.tile([128, D], F32, tag="ot", name="ot")
                                nc.vector.tensor_add(
                                    ot[:tsz], ps[:tsz, 0:D], bobc[:tsz])
                                nc.sync.dma_start(
                                    out=out_d[b, t0 : t0 + tsz, :], in_=ot[:tsz])

                        steps.append((go, fc == 0))
                return steps

            def run_all(steps):
                for s, _ in steps:
                    s()

            # ---------------- attention for item b, fillers interleaved
            def emit_attention(b, fillers):
                qk = st[("qk", b)]
                v = st[("v", b)]
                ktz = ktzs[b % 2]
                ctxT = cpool.tile([128, FC, S], BF16, tag="ctx", name="ctx")
                st[("ctx", b)] = ctxT

                nslots = H + 2
                # pop points: one after each kc iteration + one per slot end
                points = {"left": nslots * (TT + 1)}

                def pop_fill():
                    points["left"] -= 1
                    if not fillers:
                        return
                    n = (len(fillers) + max(points["left"], 1) - 1) // max(
                        points["left"], 1
                    )
                    popped = 0
                    while fillers and popped < n:
                        fn, chunk_start = fillers[0]
                        if chunk_start and popped > 0:
                            break  # new chunk starts at the NEXT pop point
                        fillers.popleft()
                        fn()
                        popped += 1

                hstate = {}  # h -> probs tile
                box = {"psc": None}
                for hs in range(nslots):
                    h_s = hs if hs < H else None
                    h_c = hs - 2 if hs >= 2 else None
                    if h_s is not None:
                        m, j = h_s // 2, h_s % 2
                        probs = ppool.tile(
                            [128, TT, S], BF16, tag="probs", name="probs"
                        )
                        den = dpool.tile([128, TT], F32, tag="den", name="den")
                        nc.vector.memset(den, 1.0)
                        hstate[h_s] = probs
                    if h_c is not None:
                        probs_c = hstate.pop(h_c) if h_c % 2 == 1 else hstate[h_c]
                        if h_c % 2 == 0:
                            box["psc"] = psc.tile(
                                [128, D], F32, tag="psc", name="pscx"
                            )
                        vsz_c = vszs[h_c % 2]
                    for kc in range(TT):
                        ksz = min(128, S - kc * 128)
                        k0 = kc * 128
                        if h_s is not None:
                            ps = pss.tile([128, D], F32, tag="pss", name="pssc")
                            nc.tensor.matmul(
                                ps[:ksz, 0:S0],
                                lhsT=ktz[:, m, j, k0 : k0 + ksz],
                                rhs=qk[:, m, 0:S0], start=True, stop=True)
                            nc.tensor.matmul(
                                ps[:ksz, S0:S],
                                lhsT=ktz[:, m, j, k0 : k0 + ksz],
                                rhs=qk[:, m, S0:S], start=True, stop=True)
                            # exp + row-sum (over q) fused in the eviction
                            nc.scalar.activation(
                                probs[:ksz, kc, :], ps[:ksz, 0:S], AF.Exp,
                                accum_out=den[:ksz, kc : kc + 1])
                        if h_c is not None:
                            stt = (h_c % 2 == 0) and kc == 0
                            spp = (h_c % 2 == 1) and kc == TT - 1
                            nc.tensor.matmul(
                                box["psc"][:, 0:S0], lhsT=vsz_c[:ksz, kc, :],
                                rhs=probs_c[:ksz, kc, 0:S0], start=stt, stop=spp)
                            nc.tensor.matmul(
                                box["psc"][:, S0:S], lhsT=vsz_c[:ksz, kc, :],
                                rhs=probs_c[:ksz, kc, S0:S], start=stt, stop=spp)
                        pop_fill()
                    if h_s is not None:
                        # fold 1/denominator into this head's V rows (GpSimd;
                        # writes only the live half of the persistent buffer)
                        po = j * 64
                        rd = dpool.tile([128, TT], F32, tag="rd", name="rd")
                        nc.vector.reciprocal(rd, den)
                        vsz = vszs[j]
                        for kc in range(TT):
                            ksz = min(128, S - kc * 128)
                            nc.vector.tensor_scalar_mul(
                                vsz[:ksz, kc, po : po + 64],
                                v[:ksz, kc, h_s * DH : (h_s + 1) * DH],
                                rd[:ksz, kc : kc + 1])
                    if h_c is not None and h_c % 2 == 1:
                        nc.vector.tensor_copy(
                            ctxT[:, h_c // 2, :], box["psc"][:, 0:S])
                    pop_fill()
                # drain any leftover fillers
                while fillers:
                    fillers.popleft()[0]()

            # ---------------- software-pipelined item loop.
            # Each item's QK projection is split: the chunks needed by the
            # first six head-slots run as fillers of the PREVIOUS item's
            # attention; the rest ("carry") run inside the item's own
            # attention, before their head-slot deadline.  This spreads
            # filler work evenly so even the last item's attention has
            # independent PE work while ScalarE drains the exp chain.
            FIRST_MS = [0, 1, 2, FC, FC + 1, FC + 2]
            CARRY_MS = [3, FC + 3, 4, FC + 4, 5, FC + 5]
            # Prologue: QK chunks to psf, V chunks to pss, alternating per
            # chunk so each pool's eviction hides under the other's matmuls.
            run_all(load_steps(0))
            if bc > 1:
                run_all(load_steps(1))
            qs, vs = qk_steps(0, FIRST_MS), v_steps(0, pool=pss)
            qchunks = [qs[i : i + DC] for i in range(0, len(qs), DC)]
            vchunks = [vs[i : i + DC] for i in range(0, len(vs), DC)]
            order = []
            for i in range(max(len(qchunks), len(vchunks))):
                if i < len(qchunks):
                    order.extend(qchunks[i])
                if i < len(vchunks):
                    order.extend(vchunks[i])
            run_all(order)
            for b in range(bc):
                fillers = deque()
                fillers.extend(qk_steps(b, CARRY_MS))  # deadline: slot 2m
                if b + 2 < bc:
                    fillers.extend(load_steps(b + 2))
                if b >= 1:
                    fillers.extend(o_steps(b - 1))
                if b + 1 < bc:
                    fillers.extend(qk_steps(b + 1, FIRST_MS))
                    fillers.extend(v_steps(b + 1))
                emit_attention(b, fillers)
            run_all(o_steps(bc - 1, alt=True))

    return nc


# ---------------------------------------------------------------- host prep
def _prep_shared(Wq, bq, Wk, bk, Wv, bv, Wo, bo):
    """Build the per-core-identical weight operands."""
    scale = np.float32(1.0 / np.sqrt(DH))
    wqf = (Wq.astype(np.float32) * scale).transpose(1, 0, 2).reshape(D, D)
    wkf = Wk.astype(np.float32).transpose(1, 0, 2).reshape(D, D)
    wvf = Wv.astype(np.float32).transpose(1, 0, 2).reshape(D, D)

    def chunk4(wf):  # [d, f] -> [di, m, dc, fi]
        return wf.reshape(DC, 128, FC, 128).transpose(1, 2, 0, 3)

    wqk = np.concatenate([chunk4(wqf), chunk4(wkf)], axis=1)  # [128, 12, 6, 128]
    wqk = wqk.transpose(2, 0, 1, 3)                           # [6, 128, 12, 128]
    wv3 = wvf.reshape(DC, 128, D).transpose(1, 0, 2)          # [128, 6, 768]
    wo3 = Wo.astype(np.float32).reshape(FC, 128, D).transpose(1, 0, 2)

    bqf = (bq.astype(np.float32) * scale).reshape(D)
    bkf = bk.astype(np.float32).reshape(D)
    bqk = np.concatenate(
        [bqf.reshape(FC, 128), bkf.reshape(FC, 128)], axis=0
    ).T.copy()                                                # [128, 12]
    bvbc = np.broadcast_to(bv.astype(np.float32).reshape(D), (128, D)).copy()
    bobc = np.broadcast_to(bo.astype(np.float32).reshape(D), (128, D)).copy()

    return {
        "wqk": np.ascontiguousarray(wqk).astype(nbf),
        "wv": np.ascontiguousarray(wv3).astype(nbf),
        "wo": np.ascontiguousarray(wo3).astype(nbf),
        "bqk": np.ascontiguousarray(bqk),
        "bvbc": bvbc,
        "bobc": bobc,
    }


_NC_CACHE = {}


def kernel(x, Wq, bq, Wk, bk, Wv, bv, Wo, bo):
    x = np.asarray(x, dtype=np.float32)
    shared = _prep_shared(
        np.asarray(Wq), np.asarray(bq), np.asarray(Wk), np.asarray(bk),
        np.asarray(Wv), np.asarray(bv), np.asarray(Wo), np.asarray(bo))

    in_maps = []
    for c in range(NCORES):
        xc = x[c * BC : (c + 1) * BC]                    # [BC, S, D]
        xt = xc.transpose(2, 0, 1)                       # [D, BC, S]
        xt = xt.reshape(DC, 128, BC, S).astype(nbf)
        m = dict(shared)
        m["xt"] = np.ascontiguousarray(xt)
        in_maps.append(m)

    if "nc" not in _NC_CACHE:
        _NC_CACHE["nc"] = build_bass()
    nc = _NC_CACHE["nc"]

    res = run_bass_kernel_spmd(nc, in_maps, core_ids=list(range(NCORES)))
    out = np.concatenate([res.results[c]["out"] for c in range(NCORES)], axis=0)
    return out.astype(np.float32)


if __name__ == "__main__":
    rng = np.random.default_rng(0)
    ins = {
        "x": rng.standard_normal((B, S, D), dtype=np.float32),
        "Wq": rng.standard_normal((H, D, DH), dtype=np.float32) * 0.02,
        "bq": np.zeros((H, DH), np.float32),
        "Wk": rng.standard_normal((H, D, DH), dtype=np.float32) * 0.02,
        "bk": np.zeros((H, DH), np.float32),
        "Wv": rng.standard_normal((H, D, DH), dtype=np.float32) * 0.02,
        "bv": np.zeros((H, DH), np.float32),
        "Wo": rng.standard_normal((D, D), dtype=np.float32) * 0.02,
        "bo": np.zeros((D,), np.float32),
    }
    o = kernel(**ins)
    print("out", o.shape, o.dtype, float(np.abs(o).max()))


# revision 16
# speedup vs baseline: 1.3667x; 1.0143x over previous
"""Trainium2 Bass kernel for nn_MultiHeadAttention_31542239822105.

Math (faithful to reference, incl. softmax over the QUERY axis):
  q = einsum('bsd,hde->bhse', x, Wq) + bq ; same k, v
  scores = q @ k^T * 1/sqrt(DH)          [B,H,Sq,Sk]
  probs  = softmax(scores, axis=2)       # over q (query axis!)
  ctx    = einsum('bhqk,bhke->bhqe', probs, v)
  out    = ctx.reshape(B,S,D) @ Wo + bo

Sharding: data-parallel over batch, 8 cores x 8 batch items. No collectives.

Per-core layout strategy (all matmul contraction dims land on partitions):
  - x is pre-transposed on the HOST to xT [D, tokens] so no on-chip transposes.
  - Q^T,K^T come out of the projection f-major ([feat, token]) with W as the
    stationary operand; V comes out token-major with xT as stationary.
  - scoresT[k,q] = K^T.T @ Q^T per head -> softmax over q is a FREE-axis
    reduction; exp+sum fused into the PSUM eviction on ScalarE (accum_out).
  - 1/denominator is folded into V rows (cheap: S*DH vs S*S elements).
  - ctxT[f,q] accumulates per head pair into one PSUM tile; output projection
    uses ctxT chunks as stationary -> token-major result, direct DMA out.
  - 1/sqrt(DH) folded into Wq/bq on the host.

Schedule (the perf-critical part): the ScalarE exp/accum chain (~830ns per
128x577 tile) is slower than the PE's scores+ctx work for the same tile
(~480ns), so a per-item serial stage order starves the PE during attention.
Instead the emission order software-pipelines ACROSS batch items: while
item b's attention drains on ScalarE, the PE executes interleaved "filler"
matmuls from item b+1's QK/V projections and item b-1's output projection.
Fillers are popped from a queue between the scores/ctx matmul pairs, so the
in-order PE stream always has independent work and stays in its fast clock
state (p-state ramps to 2.4 GHz only after ~3us of continuous execution).

Engine balance per item: PE ~65us, ScalarE(exp+accum) ~50us, DVE (all
PSUM evictions) ~15us, GpSimd (1/den folding into V) ~10us.

K^T and the scaled-V operands are zero-PADDED per head so the scores/ctx
lhsT is a full 128-partition operand (half-shape matmuls drop the PE out of
its fast clock); the zero halves live in persistent double-buffered tiles
that are memset once at kernel start, never per item.
"""

import sys
from collections import deque

if "/opt/trn_rl_repo" not in sys.path:
    sys.path.insert(0, "/opt/trn_rl_repo")

import numpy as np
import ml_dtypes

import concourse.bass as bass
import concourse.mybir as mybir
import concourse.tile as tile_mod
from concourse.vector_clock import ScopedClock
from concourse.bass_utils import run_bass_kernel_spmd

# ---------------------------------------------------------------- constants
B, S, D, H = 64, 577, 768, 12
DH = D // H          # 64
NCORES = 8
BC = B // NCORES     # 8 batch items per core
DC = D // 128        # 6 d-chunks
FC = D // 128        # 6 f-chunks per projection matrix
M_QK = 2 * FC        # 12 combined Q+K f-chunks
TT = (S + 127) // 128  # 5 token tiles (128,128,128,128,65)
S0 = 512             # PSUM-bank-sized free-dim split: 577 = 512 + 65
S1 = S - S0

BF16 = mybir.dt.bfloat16
F32 = mybir.dt.float32
nbf = ml_dtypes.bfloat16

_TILE_PATCHED = False
_CUR_NC = [None]


def _patch_tile_drain():
    """The walrus build here rejects >1 sync-wait per instruction
    ("Too many sync wait commands"). Two patches:
    1. post-legalize pass that moves extra waits onto single-wait nops
       inserted just before the offending instruction (same engine);
    2. the final SP Drain (emitted after legalize) gets the same split.
    """
    global _TILE_PATCHED
    if _TILE_PATCHED:
        return
    _TILE_PATCHED = True

    _orig_postorder = tile_mod.postorder_instruction_blocks

    def _split_multi_waits(ordered, nc):
        for bbname, insts in ordered.items():
            out = []
            n_split = 0
            for inst in insts:
                si = inst.sync_info
                if si is not None and len(si.on_wait) > 1:
                    waits = list(si.on_wait)
                    for w in waits[:-1]:
                        nop = mybir.InstNoOp(
                            name=nc.get_next_instruction_name(),
                            ins=[],
                            outs=[],
                            bass_is_fusable=False,
                        )
                        nop.engine = inst.engine
                        nop.sync_info = mybir.SyncInfo(on_wait=[w], on_update=[])
                        nc.register_instruction(nop, overwrite=True)
                        out.append(nop)
                        n_split += 1
                    inst.sync_info = mybir.SyncInfo(
                        on_wait=[waits[-1]], on_update=list(si.on_update)
                    )
                out.append(inst)
            ordered[bbname] = out
        return ordered

    def postorder_and_split(ordered, start_bb, postordered):
        # Runs post-sem-assignment, right before lowering: the only spot
        # where the final per-instruction waits are visible and editable.
        nc = _CUR_NC[0]
        _split_multi_waits(ordered, nc)
        return _orig_postorder(ordered, start_bb, postordered)

    tile_mod.postorder_instruction_blocks = postorder_and_split

    def _drain_and_barrier_split(self, tick_clock, wait_clock):
        nc = self.nc
        drain_inst = nc.sync.drain()
        wait_clock.add_sem_waits(
            drain_inst.ins, ScopedClock({None: tick_clock.global_clock})
        )
        si = drain_inst.ins.sync_info
        waits = list(si.on_wait)
        if len(waits) > 1:
            drain_inst.ins.sync_info = mybir.SyncInfo(
                on_wait=[waits[0]], on_update=list(si.on_update)
            )
            for w in waits[1:]:
                nop = nc.sync.nop(nofuse=True)
                nop.ins.sync_info = mybir.SyncInfo(on_wait=[w], on_update=[])
        nc.all_engine_barrier()
        assert self.sems is not None
        popped = nc._tile_sem_poison_stack.pop()
        assert popped is self._sem_poison
        nc.clear_and_free_semaphores(list(self.sems.allocated().values()))
        nc.all_engine_barrier()

    tile_mod.TileContext._drain_and_barrier = _drain_and_barrier_split


# ---------------------------------------------------------------- builder
def build_bass(bc=BC):
    """Emit the per-core kernel for `bc` batch items. Returns nc."""
    _patch_tile_drain()
    nc = bass.Bass()
    _CUR_NC[0] = nc

    xt_d = nc.declare_dram_parameter("xt", [128, bc, DC, S], BF16, isOutput=False)
    wqk_d = nc.declare_dram_parameter("wqk", [DC, 128, M_QK, 128], BF16, isOutput=False)
    wv_d = nc.declare_dram_parameter("wv", [128, DC, D], BF16, isOutput=False)
    wo_d = nc.declare_dram_parameter("wo", [128, FC, D], BF16, isOutput=False)
    bqk_d = nc.declare_dram_parameter("bqk", [128, M_QK], F32, isOutput=False)
    bvbc_d = nc.declare_dram_parameter("bvbc", [128, D], F32, isOutput=False)
    bobc_d = nc.declare_dram_parameter("bobc", [128, D], F32, isOutput=False)
    out_d = nc.declare_dram_parameter("out", [bc, S, D], F32, isOutput=True)

    AF = mybir.ActivationFunctionType

    with tile_mod.TileContext(nc) as tc:
        with (
            tc.tile_pool(name="singles", bufs=1) as singles,
            tc.tile_pool(name="xt", bufs=3) as xpool,
            tc.tile_pool(name="qk", bufs=2) as qkpool,
            tc.tile_pool(name="v", bufs=2) as vpool,
            tc.tile_pool(name="probs", bufs=4) as ppool,
            tc.tile_pool(name="den", bufs=4) as dpool,
            tc.tile_pool(name="ctx", bufs=3) as cpool,
            tc.tile_pool(name="ot", bufs=3) as opool,
            tc.tile_pool(name="pss", bufs=2, space="PSUM") as pss,
            tc.tile_pool(name="psc", bufs=1, space="PSUM") as psc,
            tc.tile_pool(name="psf", bufs=1, space="PSUM") as psf,
        ):
            # -------- resident weights / biases.  wqk is split per d-chunk so
            # the first projection matmul only waits on its own slice.
            wqk = singles.tile([128, DC, M_QK, 128], BF16, tag="wqk")
            for dc in range(DC):
                nc.sync.dma_start(out=wqk[:, dc, :, :], in_=wqk_d[dc, :, :, :])
            bqk = singles.tile([128, M_QK], F32, tag="bqk")
            nc.sync.dma_start(out=bqk, in_=bqk_d[:])
            wv = singles.tile([128, DC, D], BF16, tag="wv")
            nc.sync.dma_start(out=wv, in_=wv_d[:])
            wo = singles.tile([128, FC, D], BF16, tag="wo")
            nc.sync.dma_start(out=wo, in_=wo_d[:])
            bvbc = singles.tile([128, D], F32, tag="bvbc")
            nc.sync.dma_start(out=bvbc, in_=bvbc_d[:])
            bobc = singles.tile([128, D], F32, tag="bobc")
            nc.sync.dma_start(out=bobc, in_=bobc_d[:])

            # Persistent zero-padded operand buffers (double-buffered by item
            # parity for ktz, by head parity for vsz).  The zero halves are
            # written ONCE here and never touched again.
            ktzs = [
                singles.tile([128, FC, 2, S], BF16, tag=f"ktz{i}", name=f"ktz{i}")
                for i in range(2)
            ]
            vszs = [
                [
                    singles.tile(
                        [128, TT, 128], BF16, tag=f"vsz{j}{a}", name=f"vsz{j}{a}"
                    )
                    for a in range(2)
                ]
                for j in range(2)
            ]
            for i in range(2):
                for mk in range(FC):
                    nc.vector.memset(ktzs[i][64:128, mk, 0, :], 0.0)
                    nc.vector.memset(ktzs[i][0:64, mk, 1, :], 0.0)
            for a in range(2):
                nc.vector.memset(vszs[0][a][:, :, 64:128], 0.0)
                nc.vector.memset(vszs[1][a][:, :, 0:64], 0.0)

            st = {}  # cross-stage tile handles: ('xt'|'qk'|'v'|'ctx', b)

            # ---------------- filler-step generators (one PE matmul pair
            # or one DMA batch per step; evictions ride along).
            # Filler steps are (emit_fn, chunk_start) pairs.  chunk_start
            # marks the first matmul of a PSUM accumulation chunk: the
            # interleaver never lets it follow the previous chunk's last
            # step inside one pop batch, so the single-buffered psf pool's
            # eviction latency always hides under a scores/ctx pair.
            def load_steps(b):
                def go():
                    xt = xpool.tile([128, DC, S], BF16, tag="xt", name="xt")
                    st[("xt", b)] = xt
                    nc.sync.dma_start(out=xt[:, :, :], in_=xt_d[:, b, :, :])

                return [(go, False)]

            def qk_steps(b, ms=None):
                steps = []
                box = {}
                for m in ms if ms is not None else range(M_QK):
                    for dc in range(DC):
                        def go(m=m, dc=dc):
                            xt = st[("xt", b)]
                            if ("qk", b) not in st:
                                st[("qk", b)] = qkpool.tile(
                                    [128, FC, S], BF16, tag="qk", name="qk"
                                )
                            if dc == 0:
                                box["ps"] = psf.tile(
                                    [128, D], F32, tag="psf", name="psfq"
                                )
                            ps = box["ps"]
                            stt, spp = dc == 0, dc == DC - 1
                            nc.tensor.matmul(
                                ps[:, 0:S0], lhsT=wqk[:, dc, m, :],
                                rhs=xt[:, dc, 0:S0], start=stt, stop=spp)
                            nc.tensor.matmul(
                                ps[:, S0:S], lhsT=wqk[:, dc, m, :],
                                rhs=xt[:, dc, S0:S], start=stt, stop=spp)
                            if spp:
                                if m < FC:
                                    nc.vector.tensor_scalar_add(
                                        st[("qk", b)][:, m, :], ps[:, 0:S],
                                        bqk[:, m : m + 1])
                                else:
                                    mk = m - FC
                                    ktz = ktzs[b % 2]
                                    nc.vector.tensor_scalar_add(
                                        ktz[0:64, mk, 0, :], ps[0:64, 0:S],
                                        bqk[0:64, m : m + 1])
                                    nc.vector.tensor_scalar_add(
                                        ktz[64:128, mk, 1, :], ps[64:128, 0:S],
                                        bqk[64:128, m : m + 1])

                        steps.append((go, dc == 0))
                    if m >= FC:
                        # K evictions are 2 DVE ops (~1.6us): charge one
                        # extra pop point before the next chunk may start
                        steps.append((lambda: None, True))
                return steps

            def v_steps(b, pool=None):
                steps = []
                box = {}
                for tt in range(TT):
                    for dc in range(DC):
                        def go(tt=tt, dc=dc):
                            xt = st[("xt", b)]
                            tsz = min(128, S - tt * 128)
                            t0 = tt * 128
                            if tt == 0 and dc == 0:
                                st[("v", b)] = vpool.tile(
                                    [128, TT, D], BF16, tag="v", name="v"
                                )
                            if dc == 0:
                                p = pool if pool is not None else psf
                                tg = "pss" if pool is not None else "psf"
                                box["ps"] = p.tile(
                                    [128, D], F32, tag=tg, name="psfv"
                                )
                            ps = box["ps"]
                            stt, spp = dc == 0, dc == DC - 1
                            nc.tensor.matmul(
                                ps[:tsz, 0:S0], lhsT=xt[:, dc, t0 : t0 + tsz],
                                rhs=wv[:, dc, 0:S0], start=stt, stop=spp)
                            nc.tensor.matmul(
                                ps[:tsz, S0:D], lhsT=xt[:, dc, t0 : t0 + tsz],
                                rhs=wv[:, dc, S0:D], start=stt, stop=spp)
                            if spp:
                                nc.vector.tensor_add(
                                    st[("v", b)][:tsz, tt, :], ps[:tsz, 0:D],
                                    bvbc[:tsz])

                        steps.append((go, dc == 0))
                return steps

            def o_steps(b, alt=False):
                steps = []
                box = {}
                for tt in range(TT):
                    for fc in range(FC):
                        def go(tt=tt, fc=fc):
                            ctxT = st[("ctx", b)]
                            tsz = min(128, S - tt * 128)
                            t0 = tt * 128
                            if fc == 0:
                                # alt: alternate chunks between psf and the
                                # (idle at epilogue) pss pool so the single
                                # psf buffer's eviction latency never stalls
                                # back-to-back chunks
                                if alt and tt % 2 == 1:
                                    box["ps"] = pss.tile(
                                        [128, D], F32, tag="pss", name="psso"
                                    )
                                else:
                                    box["ps"] = psf.tile(
                                        [128, D], F32, tag="psf", name="psfo"
                                    )
                            ps = box["ps"]
                            stt, spp = fc == 0, fc == FC - 1
                            nc.tensor.matmul(
                                ps[:tsz, 0:S0], lhsT=ctxT[:, fc, t0 : t0 + tsz],
                                rhs=wo[:, fc, 0:S0], start=stt, stop=spp)
                            nc.tensor.matmul(
                                ps[:tsz, S0:D], lhsT=ctxT[:, fc, t0 : t0 + tsz],
                                rhs=wo[:, fc, S0:D], start=stt, stop=spp)
                            if spp:
                                ot = opool.tile([128, D], F32, tag="ot", name="ot")
                                nc.vector.tensor_add(
                                    ot[:tsz], ps[:tsz, 0:D], bobc[:tsz])
                                nc.sync.dma_start(
                                    out=out_d[b, t0 : t0 + tsz, :], in_=ot[:tsz])

                        steps.append((go, fc == 0))
                return steps

            def run_all(steps):
                for s, _ in steps:
                    s()

            # ---------------- attention for item b, fillers interleaved
            def emit_attention(b, fillers):
                qk = st[("qk", b)]
                v = st[("v", b)]
                ktz = ktzs[b % 2]
                ctxT = cpool.tile([128, FC, S], BF16, tag="ctx", name="ctx")
                st[("ctx", b)] = ctxT

                nslots = H + 2
                # pop points: one after each kc iteration + one per slot end
                points = {"left": nslots * (TT + 1)}

                def pop_fill():
                    points["left"] -= 1
                    if not fillers:
                        return
                    n = (len(fillers) + max(points["left"], 1) - 1) // max(
                        points["left"], 1
                    )
                    popped = 0
                    while fillers and popped < n:
                        fn, chunk_start = fillers[0]
                        if chunk_start and popped > 0:
                            break  # new chunk starts at the NEXT pop point
                        fillers.popleft()
                        fn()
                        popped += 1

                hstate = {}  # h -> probs tile
                box = {"psc": None}
                for hs in range(nslots):
                    h_s = hs if hs < H else None
                    h_c = hs - 2 if hs >= 2 else None
                    if h_s is not None:
                        m, j = h_s // 2, h_s % 2
                        probs = ppool.tile(
                            [128, TT, S], BF16, tag="probs", name="probs"
                        )
                        den = dpool.tile([128, TT], F32, tag="den", name="den")
                        hstate[h_s] = probs
                    if h_c is not None:
                        probs_c = hstate.pop(h_c) if h_c % 2 == 1 else hstate[h_c]
                        if h_c % 2 == 0:
                            box["psc"] = psc.tile(
                                [128, D], F32, tag="psc", name="pscx"
                            )
                        vsz_c = vszs[h_c % 2][(h_c // 2) % 2]
                    for kc in range(TT):
                        ksz = min(128, S - kc * 128)
                        k0 = kc * 128
                        if h_s is not None:
                            ps = pss.tile([128, D], F32, tag="pss", name="pssc")
                            nc.tensor.matmul(
                                ps[:ksz, 0:S0],
                                lhsT=ktz[:, m, j, k0 : k0 + ksz],
                                rhs=qk[:, m, 0:S0], start=True, stop=True)
                            nc.tensor.matmul(
                                ps[:ksz, S0:S],
                                lhsT=ktz[:, m, j, k0 : k0 + ksz],
                                rhs=qk[:, m, S0:S], start=True, stop=True)
                            # exp + row-sum (over q) fused in the eviction
                            nc.scalar.activation(
                                probs[:ksz, kc, :], ps[:ksz, 0:S], AF.Exp,
                                accum_out=den[:ksz, kc : kc + 1])
                        if h_c is not None:
                            stt = (h_c % 2 == 0) and kc == 0
                            spp = (h_c % 2 == 1) and kc == TT - 1
                            nc.tensor.matmul(
                                box["psc"][:, 0:S0], lhsT=vsz_c[:ksz, kc, :],
                                rhs=probs_c[:ksz, kc, 0:S0], start=stt, stop=spp)
                            nc.tensor.matmul(
                                box["psc"][:, S0:S], lhsT=vsz_c[:ksz, kc, :],
                                rhs=probs_c[:ksz, kc, S0:S], start=stt, stop=spp)
                        pop_fill()
                    if h_s is not None:
                        # fold 1/denominator into this head's V rows (GpSimd;
                        # writes only the live half of the persistent buffer)
                        po = j * 64
                        rd = dpool.tile([128, TT], F32, tag="rd", name="rd")
                        nc.vector.reciprocal(rd, den)
                        vsz = vszs[j][(h_s // 2) % 2]
                        for kc in range(TT):
                            ksz = min(128, S - kc * 128)
                            nc.vector.tensor_scalar_mul(
                                vsz[:ksz, kc, po : po + 64],
                                v[:ksz, kc, h_s * DH : (h_s + 1) * DH],
                                rd[:ksz, kc : kc + 1])
                    if h_c is not None and h_c % 2 == 1:
                        nc.vector.tensor_copy(
                            ctxT[:, h_c // 2, :], box["psc"][:, 0:S])
                    pop_fill()
                # drain any leftover fillers
                while fillers:
                    fillers.popleft()[0]()

            # ---------------- software-pipelined item loop.
            # Each item's QK projection is split: the chunks needed by the
            # first six head-slots run as fillers of the PREVIOUS item's
            # attention; the rest ("carry") run inside the item's own
            # attention, before their head-slot deadline.  This spreads
            # filler work evenly so even the last item's attention has
            # independent PE work while ScalarE drains the exp chain.
            FIRST_MS = [0, 1, 2, FC, FC + 1, FC + 2]
            CARRY_MS = [3, FC + 3, 4, FC + 4, 5, FC + 5]
            # Prologue: QK chunks to psf, V chunks to pss, alternating per
            # chunk so each pool's eviction hides under the other's matmuls.
            run_all(load_steps(0))
            if bc > 1:
                run_all(load_steps(1))
            ks0 = qk_steps(0, [FC, FC + 1, FC + 2])
            qs0 = qk_steps(0, [0, 1, 2])
            vs0 = v_steps(0, pool=pss)
            kchunks = [ks0[i : i + DC + 1] for i in range(0, len(ks0), DC + 1)]
            qchunks = [qs0[i : i + DC] for i in range(0, len(qs0), DC)]
            vchunks = [vs0[i : i + DC] for i in range(0, len(vs0), DC)]
            order = []
            for i in range(max(len(kchunks), len(qchunks), len(vchunks))):
                if i < len(kchunks):
                    order.extend(kchunks[i])
                if i < len(qchunks):
                    order.extend(qchunks[i])
                if i < len(vchunks):
                    order.extend(vchunks[i])
            run_all(order)
            for b in range(bc):
                fillers = deque()
                fillers.extend(qk_steps(b, CARRY_MS))  # deadline: slot 2m
                if b + 2 < bc:
                    fillers.extend(load_steps(b + 2))
                if b >= 1:
                    fillers.extend(o_steps(b - 1))
                if b + 1 < bc:
                    fillers.extend(qk_steps(b + 1, FIRST_MS))
                    fillers.extend(v_steps(b + 1))
                emit_attention(b, fillers)
            run_all(o_steps(bc - 1, alt=True))

    return nc


# ---------------------------------------------------------------- host prep
def _prep_shared(Wq, bq, Wk, bk, Wv, bv, Wo, bo):
    """Build the per-core-identical weight operands."""
    scale = np.float32(1.0 / np.sqrt(DH))
    wqf = (Wq.astype(np.float32) * scale).transpose(1, 0, 2).reshape(D, D)
    wkf = Wk.astype(np.float32).transpose(1, 0, 2).reshape(D, D)
    wvf = Wv.astype(np.float32).transpose(1, 0, 2).reshape(D, D)

    def chunk4(wf):  # [d, f] -> [di, m, dc, fi]
        return wf.reshape(DC, 128, FC, 128).transpose(1, 2, 0, 3)

    wqk = np.concatenate([chunk4(wqf), chunk4(wkf)], axis=1)  # [128, 12, 6, 128]
    wqk = wqk.transpose(2, 0, 1, 3)                           # [6, 128, 12, 128]
    wv3 = wvf.reshape(DC, 128, D).transpose(1, 0, 2)          # [128, 6, 768]
    wo3 = Wo.astype(np.float32).reshape(FC, 128, D).transpose(1, 0, 2)

    bqf = (bq.astype(np.float32) * scale).reshape(D)
    bkf = bk.astype(np.float32).reshape(D)
    bqk = np.concatenate(
        [bqf.reshape(FC, 128), bkf.reshape(FC, 128)], axis=0
    ).T.copy()                                                # [128, 12]
    bvbc = np.broadcast_to(bv.astype(np.float32).reshape(D), (128, D)).copy()
    bobc = np.broadcast_to(bo.astype(np.float32).reshape(D), (128, D)).copy()

    return {
        "wqk": np.ascontiguousarray(wqk).astype(nbf),
        "wv": np.ascontiguousarray(wv3).astype(nbf),
        "wo": np.ascontiguousarray(wo3).astype(nbf),
        "bqk": np.ascontiguousarray(bqk),
        "bvbc": bvbc,
        "bobc": bobc,
    }


_NC_CACHE = {}


def kernel(x, Wq, bq, Wk, bk, Wv, bv, Wo, bo):
    x = np.asarray(x, dtype=np.float32)
    shared = _prep_shared(
        np.asarray(Wq), np.asarray(bq), np.asarray(Wk), np.asarray(bk),
        np.asarray(Wv), np.asarray(bv), np.asarray(Wo), np.asarray(bo))

    in_maps = []
    for c in range(NCORES):
        xc = x[c * BC : (c + 1) * BC]                    # [BC, S, D]
        xt = xc.transpose(2, 0, 1)                       # [D, BC, S]
        xt = xt.reshape(DC, 128, BC, S).transpose(1, 2, 0, 3).astype(nbf)
        m = dict(shared)
        m["xt"] = np.ascontiguousarray(xt)
        in_maps.append(m)

    if "nc" not in _NC_CACHE:
        _NC_CACHE["nc"] = build_bass()
    nc = _NC_CACHE["nc"]

    res = run_bass_kernel_spmd(nc, in_maps, core_ids=list(range(NCORES)))
    out = np.concatenate([res.results[c]["out"] for c in range(NCORES)], axis=0)
    return out.astype(np.float32)


if __name__ == "__main__":
    rng = np.random.default_rng(0)
    ins = {
        "x": rng.standard_normal((B, S, D), dtype=np.float32),
        "Wq": rng.standard_normal((H, D, DH), dtype=np.float32) * 0.02,
        "bq": np.zeros((H, DH), np.float32),
        "Wk": rng.standard_normal((H, D, DH), dtype=np.float32) * 0.02,
        "bk": np.zeros((H, DH), np.float32),
        "Wv": rng.standard_normal((H, D, DH), dtype=np.float32) * 0.02,
        "bv": np.zeros((H, DH), np.float32),
        "Wo": rng.standard_normal((D, D), dtype=np.float32) * 0.02,
        "bo": np.zeros((D,), np.float32),
    }
    o = kernel(**ins)
    print("out", o.shape, o.dtype, float(np.abs(o).max()))
